# revision 30
# baseline (speedup 1.0000x reference)
"""Trainium2 Bass kernel for nn_MDRMWithCPRecon.

Sharding: pure data parallel over batch B=8 -> one batch element per
NeuronCore (8 cores). All parameters replicated.

v2 changes vs baseline (294us):
  - bf16 conv (FWL weight loads, 216ns/MM vs fp32r 239ns), bf16
    everywhere downstream (DVE 2x elementwise, halved output DMA).
  - input staged in 16 quarter-chunks, converted f32->bf16 on
    scalar/vector engines, conv starts after first chunk (~7us vs 35us).
  - Fm kept in SBUF (kills the 8MB DRAM round trip).
  - U_gen folded host-side: u = (Wu@Wa) @ [avg;max] + (Wu@ba+bu) -> the
    whole adapter stage disappears.
  - recon bias br folded as a 5th row of G / MT.
  - final stage: ct-pairs processed in single wide ops, spread across
    scalar (Q, R, sig*spectral), vector (D, E, fu, E2) and gpsimd (cp).
"""

import numpy as np
import ml_dtypes

import concourse.bacc as bacc
import concourse.bass as bass
import concourse.tile as tile
from concourse import mybir, bass_utils

F32 = mybir.dt.float32
BF16 = mybir.dt.bfloat16
AF = mybir.ActivationFunctionType
ALU = mybir.AluOpType
AX = mybir.AxisListType

B, C, H, W, K = 8, 256, 64, 64, 4
HW = H * W
NCORES = 8


def build_program(alpha, ws, bs):
    from concourse.masks import make_identity

    nc = bacc.Bacc("TRN2", target_bir_lowering=False, debug=False,
                   num_devices=NCORES)

    frm = nc.dram_tensor("frm", [C, H, W], F32, kind="ExternalInput")
    oth = nc.dram_tensor("oth", [C, H, W], F32, kind="ExternalInput")
    w3t_d = nc.dram_tensor("w3t", [128, 4, 9, 256], BF16, kind="ExternalInput")
    b3_d = nc.dram_tensor("b3", [128, 2], F32, kind="ExternalInput")
    wum_d = nc.dram_tensor("wum", [1, 3, 2, 4], F32, kind="ExternalInput")
    bum_d = nc.dram_tensor("bum", [4, 3], F32, kind="ExternalInput")
    wub1_d = nc.dram_tensor("wub1", [1, 12], F32, kind="ExternalInput")
    wrt_d = nc.dram_tensor("wrt", [128, 2, 2, 128], F32, kind="ExternalInput")
    brn_d = nc.dram_tensor("brn", [128, 2], F32, kind="ExternalInput")
    wsc_d = nc.dram_tensor("wsc", [128, 4, 2, 128], F32, kind="ExternalInput")
    bsc_d = nc.dram_tensor("bsc", [128, 2], F32, kind="ExternalInput")
    lam_d = nc.dram_tensor("lam", [4, 1], F32, kind="ExternalInput")
    fused_o = nc.dram_tensor("fused", [C, H, W], BF16, kind="ExternalOutput")
    cpr_o = nc.dram_tensor("cpr", [C, H, W], BF16, kind="ExternalOutput")

    with tile.TileContext(nc) as tc:
        _build_tile(tc, nc, make_identity, locals(), alpha, ws, bs)
    nc.compile()
    return nc


def _build_tile(tc, nc, make_identity, T, alpha, ws, bs):
    frm, oth = T["frm"], T["oth"]
    w3t_d, b3_d, wum_d, bum_d = T["w3t_d"], T["b3_d"], T["wum_d"], T["bum_d"]
    wub1_d = T["wub1_d"]
    wrt_d, brn_d, wsc_d, bsc_d = (T["wrt_d"], T["brn_d"], T["wsc_d"],
                                  T["bsc_d"])
    lam_d, fused_o, cpr_o = T["lam_d"], T["fused_o"], T["cpr_o"]

    import contextlib
    ctx = contextlib.ExitStack()
    consts = ctx.enter_context(tc.tile_pool(name="consts", bufs=1))
    stage = ctx.enter_context(tc.tile_pool(name="stage", bufs=4))
    ew = ctx.enter_context(tc.tile_pool(name="ew", bufs=2))
    outr = ctx.enter_context(tc.tile_pool(name="outr", bufs=2))
    ps_conv = ctx.enter_context(tc.tile_pool(name="ps_conv", bufs=2, space="PSUM"))
    ps_sm = ctx.enter_context(tc.tile_pool(name="ps_sm", bufs=2, space="PSUM"))
    ps_spat = ctx.enter_context(tc.tile_pool(name="ps_spat", bufs=2, space="PSUM"))
    ps_rec = ctx.enter_context(tc.tile_pool(name="ps_rec", bufs=1, space="PSUM"))

    # ---- conv weights first on the scalar queue (kt0 gates first matmul) --
    w3_sb = consts.tile([128, 4, 9, 256], BF16)
    for kt in range(4):
        nc.scalar.dma_start(w3_sb[:, kt], w3t_d[:, kt])

    # ---- small weights to SBUF (sync queue) ----
    b3_sb = consts.tile([128, 2], F32)
    nc.sync.dma_start(b3_sb[:], b3_d[:])
    wum_sb = consts.tile([1, 3, 2, 4], F32)
    nc.sync.dma_start(wum_sb[:], wum_d[:])
    bum_sb = consts.tile([4, 3], F32)
    nc.sync.dma_start(bum_sb[:], bum_d[:])
    wub1_sb = consts.tile([1, 12], F32)
    nc.sync.dma_start(wub1_sb[:], wub1_d[:])
    wrt_sb = consts.tile([128, 2, 2, 128], F32)
    nc.sync.dma_start(wrt_sb[:], wrt_d[:])
    wsc_sb = consts.tile([128, 4, 2, 128], F32)
    nc.sync.dma_start(wsc_sb[:], wsc_d[:])
    bsc_sb = consts.tile([128, 2], F32)
    nc.sync.dma_start(bsc_sb[:], bsc_d[:])
    lam_sb = consts.tile([4, 1], F32)
    nc.sync.dma_start(lam_sb[:], lam_d[:])

    MT = consts.tile([4, 256], BF16)
    brn_sb = consts.tile([128, 2], F32)
    nc.sync.dma_start(brn_sb[:], brn_d[:])

    ident = consts.tile([128, 128], F32)
    make_identity(nc, ident[:])
    ones128 = consts.tile([128, 1], F32)
    nc.gpsimd.memset(ones128[:], 1.0)
    ones4 = consts.tile([4, 128], BF16)
    nc.gpsimd.memset(ones4[:], 1.0)
    ones_row = consts.tile([4, 128], F32)
    nc.gpsimd.memset(ones_row[:], 1.0)
    G = consts.tile([4, HW], BF16)          # CP factor outer products

    # warm-ups: preload ACT tables + gpsimd tensor_tensor ucode off the
    # critical path (they run during the initial DMA wait)
    scrap = consts.tile([1, 8], F32)
    nc.vector.memset(scrap[:], 0.0)
    scrap2 = consts.tile([1, 8], F32)
    nc.scalar.activation(scrap2[0:1, 0:2], scrap[0:1, 0:2], AF.Lrelu,
                         alpha=0.01)
    nc.scalar.activation(scrap2[0:1, 2:4], scrap[0:1, 2:4], AF.Exp)
    nc.scalar.activation(scrap2[0:1, 4:6], scrap[0:1, 4:6], AF.Sigmoid)
    nc.scalar.activation(scrap2[0:1, 6:8], scrap[0:1, 6:8], AF.Identity)
    scrap3 = consts.tile([4, 8], BF16)
    nc.gpsimd.tensor_tensor(scrap3[:], ones4[:, 0:8], ones4[:, 8:16],
                            op=ALU.add)

    # ---- padded bf16 image [128, 4(kt), 66, 66]; kt 0/1 frm, 2/3 oth ----
    xr = consts.tile([128, 4, 66, 66], BF16)
    for kt in range(4):
        nc.vector.memset(xr[:, kt, 0:1, :], 0.0)
        nc.vector.memset(xr[:, kt, 65:66, :], 0.0)
        nc.vector.memset(xr[:, kt, 1:65, 0:1], 0.0)
        nc.vector.memset(xr[:, kt, 1:65, 65:66], 0.0)

    # stream input in 16 quarter-chunks over the 3 DMA-capable queues
    # (sync/gpsimd early; scalar queue is busy with w3), convert
    # f32->bf16 on ACT (10) / DVE (6)
    n_cv = 0
    for q in range(4):
        for kt in range(4):
            src = frm if kt < 2 else oth
            kt2 = kt % 2
            src_v = src.rearrange("(k p) h w -> p k h w", p=128)
            stg = stage.tile([128, 16, 64], F32, tag="stg")
            if q < 2:
                eng = nc.sync if n_cv % 2 == 0 else nc.gpsimd
            else:
                eng = (nc.sync, nc.gpsimd, nc.scalar)[n_cv % 3]
            eng.dma_start(stg[:], src_v[:, kt2, q * 16:(q + 1) * 16, :])
            dst = xr[:, kt, 1 + q * 16: 1 + (q + 1) * 16, 1:65]
            if n_cv % 8 < 5:
                nc.scalar.copy(dst, stg[:])
            else:
                nc.vector.tensor_copy(dst, stg[:])
            n_cv += 1

    # ---- pooled-stat tiles ----
    sums1 = consts.tile([128, 2, 8], F32)       # per-(ct,pt) sums of Fm
    q_sum = consts.tile([128, 2, 64], F32)      # sum over w  -> [c, h]
    q_max = consts.tile([128, 2, 64], F32)
    # ping-pong accumulators for the over-h stats (combined during conv)
    ppA = consts.tile([128, 2, 2, 64], F32)     # [sum/max, ct, w] bank A
    ppB = consts.tile([128, 2, 2, 64], F32)
    fm_sb = consts.tile([128, 2, 8, 512], BF16)  # Fm resident in SBUF
    Qall = consts.tile([128, 2, 8, 512], BF16)   # (1-a)*oth per tile
    Dall = consts.tile([128, 2, 8, 512], BF16)   # a*frm - Q per tile

    a = float(alpha)

    # ---- conv3x3 + leaky relu + streaming stats + Q/D precompute ----
    for pt in range(8):
        for ct in range(2):
            ps = ps_conv.tile([128, 512], F32, tag="conv")
            idx = 0
            for kt in range(4):
                for t in range(9):
                    dy, dx = t // 3, t % 3
                    nc.tensor.matmul(
                        ps[:],
                        w3_sb[:, kt, t, ct * 128:(ct + 1) * 128],
                        xr[:, kt, pt * 8 + dy: pt * 8 + dy + 8, dx: dx + 64],
                        start=(idx == 0), stop=(idx == 35))
                    idx += 1
            nc.scalar.activation(fm_sb[:, ct, pt], ps[:], AF.Lrelu,
                                 bias=b3_sb[:, ct:ct + 1], alpha=0.01,
                                 accum_out=sums1[:, ct, pt:pt + 1])
            blk = fm_sb[:, ct, pt].rearrange("p (h w) -> p h w", h=8)
            blk_t = fm_sb[:, ct, pt].rearrange("p (h w) -> p w h", h=8)
            nc.vector.tensor_reduce(q_sum[:, ct, pt * 8:(pt + 1) * 8], blk,
                                    axis=AX.X, op=ALU.add)
            nc.vector.tensor_reduce(q_max[:, ct, pt * 8:(pt + 1) * 8], blk,
                                    axis=AX.X, op=ALU.max)
            src_pp, dst_pp = (ppA, ppB) if pt % 2 else (ppB, ppA)
            if pt == 0:
                nc.vector.tensor_reduce(ppA[:, 0, ct, :], blk_t,
                                        axis=AX.X, op=ALU.add)
                nc.vector.tensor_reduce(ppA[:, 1, ct, :], blk_t,
                                        axis=AX.X, op=ALU.max)
            else:
                ppt = ew.tile([128, 2, 64], F32, tag="ppt")
                nc.vector.tensor_reduce(ppt[:, 0, :], blk_t,
                                        axis=AX.X, op=ALU.add)
                nc.vector.tensor_reduce(ppt[:, 1, :], blk_t,
                                        axis=AX.X, op=ALU.max)
                nc.vector.tensor_tensor(dst_pp[:, 0, ct, :],
                                        src_pp[:, 0, ct, :], ppt[:, 0, :],
                                        op=ALU.add)
                nc.vector.tensor_tensor(dst_pp[:, 1, ct, :],
                                        src_pp[:, 1, ct, :], ppt[:, 1, :],
                                        op=ALU.max)
            # Q/D for the final stage (only need xr; hide under conv)
            oth_t = xr[:, 2 + ct, 1 + pt * 8: 9 + pt * 8, 1:65]
            frm_t = xr[:, ct, 1 + pt * 8: 9 + pt * 8, 1:65]
            nc.scalar.activation(
                Qall[:, ct, pt].rearrange("p (h w) -> p h w", h=8), oth_t,
                AF.Copy, scale=float(1.0 - a))
            nc.vector.scalar_tensor_tensor(
                Dall[:, ct, pt].rearrange("p (h w) -> p h w", h=8), frm_t, a,
                Qall[:, ct, pt].rearrange("p (h w) -> p h w", h=8),
                op0=ALU.mult, op1=ALU.subtract)

    # ---- combine remaining partials (pp ended in ppB after 7 adds) ----
    p_sum = ppB[:, 0]       # [128, 2, 64]
    p_max = ppB[:, 1]
    sm1 = consts.tile([128, 4], F32)    # [sum1 ct0, ct1, max1 ct0, ct1]
    nc.vector.tensor_reduce(sm1[:, 0:2], sums1[:], axis=AX.X, op=ALU.add)
    nc.vector.tensor_reduce(sm1[:, 2:4], q_max[:], axis=AX.X, op=ALU.max)

    # ---- pooled rows (modes 2/3): rows_sb[0, s, :] = [m2(64) | m3(64)] ----
    rows_sb = consts.tile([1, 2, 128], F32)

    def colrows(S_sum, S_max, denom, off):
        ssum = ps_sm.tile([1, 64], F32, tag="sm")
        nc.tensor.matmul(ssum[:], ones128[:], S_sum[:, 0, :], start=True,
                         stop=False)
        nc.tensor.matmul(ssum[:], ones128[:], S_sum[:, 1, :], start=False,
                         stop=True)
        nc.scalar.mul(rows_sb[0:1, 0, off:off + 64], ssum[:], 1.0 / denom)
        tpm = ps_sm.tile([128, 128], F32, tag="sm")
        nc.tensor.transpose(tpm[:], S_max[:].rearrange("p c w -> p (c w)"),
                            ident[:])
        mxr = ew.tile([128, 1], F32, tag="mxr")
        nc.vector.tensor_reduce(mxr[:], tpm[:], axis=AX.X, op=ALU.max)
        tpb = ps_sm.tile([1, 128], F32, tag="sm")
        nc.tensor.transpose(tpb[:], mxr[:], ident[:])
        nc.vector.tensor_reduce(
            rows_sb[0:1, 1, off:off + 64],
            tpb[:].rearrange("p (c w) -> p w c", c=2), axis=AX.X, op=ALU.max)

    colrows(p_sum, p_max, C * H, 0)
    colrows(q_sum, q_max, C * W, 64)

    U1T = consts.tile([4, 256], BF16)
    uct = consts.tile([4, 128], BF16)    # [U2T(64) | U3T(64)]
    U1n = consts.tile([128, 2, 4], F32)

    def softmax4(src, cw, dst_n):
        """src [cw, 4] logits -> dst_n [cw, 4] softmax; returns dst_n."""
        negm = ew.tile([cw, 1], F32, tag="negm")
        nc.vector.tensor_reduce(negm[:], src, axis=AX.X, op=ALU.max,
                                negate=True)
        ssum = ew.tile([cw, 1], F32, tag="ssum")
        etile = ew.tile([cw, 4], F32, tag="etile")
        nc.scalar.activation(etile[:], src, AF.Exp, bias=negm[:],
                             accum_out=ssum[:])
        rec = ew.tile([cw, 1], F32, tag="rec")
        nc.vector.reciprocal(rec[:], ssum[:])
        nc.vector.tensor_scalar(dst_n, etile[:], rec[:], None, op0=ALU.mult)
        return dst_n

    # mode 1: logits computed directly channel-on-partition:
    # u1t[c, r] = wb0[r]*avg1[c] + wb1[r]*max1[c] + bum1[r]
    bc_ps = ps_sm.tile([128, 12], F32, tag="sm")
    nc.tensor.matmul(bc_ps[:], ones_row[0:1, :], wub1_sb[:], start=True,
                     stop=True)
    bc_sb = consts.tile([128, 12], F32)
    nc.scalar.mul(bc_sb[:], bc_ps[:], 1.0)
    for ct in range(2):
        s1 = ew.tile([128, 4], F32, tag="s1")
        nc.vector.scalar_tensor_tensor(s1[:], bc_sb[:, 0:4],
                                       sm1[:, ct:ct + 1], bc_sb[:, 8:12],
                                       op0=ALU.mult, op1=ALU.add)
        u1t = ew.tile([128, 4], F32, tag="u1t")
        nc.vector.scalar_tensor_tensor(u1t[:], bc_sb[:, 4:8],
                                       sm1[:, 2 + ct:3 + ct], s1[:],
                                       op0=ALU.mult, op1=ALU.add)
        softmax4(u1t[:], 128, U1n[:, ct, :])
        tb_ps = ps_sm.tile([4, 128], F32, tag="sm")
        nc.tensor.transpose(tb_ps[:], U1n[:, ct, :], ident[:])
        nc.scalar.copy(U1T[:, ct * 128:(ct + 1) * 128], tb_ps[:])

    # modes 2/3: rank-1 matmuls from pooled rows, transpose, softmax
    for m, off in ((1, 0), (2, 64)):
        u_ps = ps_sm.tile([4, 64], F32, tag="sm")
        nc.tensor.matmul(u_ps[:], wum_sb[0:1, m, 0, :],
                         rows_sb[0:1, 0, off:off + 64], start=True, stop=False)
        nc.tensor.matmul(u_ps[:], wum_sb[0:1, m, 1, :],
                         rows_sb[0:1, 1, off:off + 64], start=False, stop=True)
        u_sb = consts.tile([4, 64], F32, tag=f"u{m}")
        nc.scalar.activation(u_sb[:], u_ps[:], AF.Identity,
                             bias=bum_sb[:, m:m + 1])
        ut_ps = ps_sm.tile([64, 4], F32, tag="sm")
        nc.tensor.transpose(ut_ps[:], u_sb[:], ident[0:4, 0:4])
        sm_t = ew.tile([64, 4], F32, tag="smt")
        softmax4(ut_ps[:], 64, sm_t[:])
        tb_ps = ps_sm.tile([4, 64], F32, tag="sm")
        nc.tensor.transpose(tb_ps[:], sm_t[:], ident[0:64, 0:64])
        nc.scalar.copy(uct[:, (m - 1) * 64:m * 64], tb_ps[:])

    # ---- G: G[r, h, w] = U3T[r,h]*U2T[r,w] (DVE/GPS halves) ----
    Gv = G[:].rearrange("p (h w) -> p h w", h=64)
    nc.vector.tensor_tensor(
        Gv[:, 0:32, :],
        uct[0:4, 64:96][:, :, None].broadcast_to([4, 32, 64]),
        uct[0:4, 0:64][:, None, :].broadcast_to([4, 32, 64]), op=ALU.mult)
    nc.gpsimd.tensor_tensor(
        Gv[:, 32:64, :],
        uct[0:4, 96:128][:, :, None].broadcast_to([4, 32, 64]),
        uct[0:4, 0:64][:, None, :].broadcast_to([4, 32, 64]), op=ALU.mult)

    # ---- MT rows 0-3 = (Wr @ U1 diag(lam))^T ----
    for mm in range(2):
        m_ps = ps_sm.tile([128, 4], F32, tag="sm")
        for kk in range(2):
            nc.tensor.matmul(m_ps[:], wrt_sb[:, kk, mm, :], U1n[:, kk, :],
                             start=(kk == 0), stop=(kk == 1))
        m_sb = ew.tile([128, 4], F32, tag="msb")
        nc.scalar.copy(m_sb[:], m_ps[:])
        mt_ps = ps_sm.tile([4, 128], F32, tag="sm")
        nc.tensor.transpose(mt_ps[:], m_sb[:], ident[:])
        nc.vector.tensor_scalar(MT[:, mm * 128:(mm + 1) * 128], mt_ps[:],
                                lam_sb[:], None, op0=ALU.mult)

    # ---- spectral attention ----
    gag = consts.tile([128, 4], F32)  # [ga_ct0, ga_ct1, gm_ct0, gm_ct1]
    for ct in range(2):
        f_ps = ps_sm.tile([128, 128], F32, tag="sm")
        nc.tensor.matmul(f_ps[:], U1T[:, ct * 128:(ct + 1) * 128], uct[:],
                         start=True, stop=True)
        nc.vector.tensor_reduce(gag[:, ct:ct + 1], f_ps[:], axis=AX.X,
                                op=ALU.add)
        nc.vector.tensor_reduce(gag[:, 2 + ct:3 + ct], f_ps[:], axis=AX.X,
                                op=ALU.max)
    spectral = consts.tile([128, 2], F32)
    for mm in range(2):
        sp_ps = ps_sm.tile([128, 1], F32, tag="sm")
        for kk in range(4):
            nc.tensor.matmul(sp_ps[:], wsc_sb[:, kk, mm, :],
                             gag[:, kk:kk + 1], start=(kk == 0),
                             stop=(kk == 3))
        stmp = ew.tile([128, 1], F32, tag="stmp")
        nc.scalar.activation(stmp[:], sp_ps[:], AF.Sigmoid,
                             bias=bsc_sb[:, mm:mm + 1])
        nc.scalar.activation(spectral[:, mm:mm + 1], stmp[:], AF.Sigmoid)

    # ---- final elementwise stage ----
    # fused = (D*spc)*sig + Q;  cp_recon = (spc*rc + spc*br)*sig + Fm
    brs = consts.tile([128, 2], F32)
    nc.vector.tensor_tensor(brs[:], brn_sb[:], spectral[:], op=ALU.mult)
    fo_v = fused_o.rearrange("(c p) h w -> p c h w", p=128)
    co_v = cpr_o.rearrange("(c p) h w -> p c h w", p=128)
    for pt in range(8):
        sp_ps = ps_spat.tile([128, 512], F32, tag="spat")
        nc.tensor.matmul(sp_ps[:], ones4[:], G[:, pt * 512:(pt + 1) * 512],
                         start=True, stop=True)
        sig = ew.tile([128, 512], BF16, tag="sig")
        nc.scalar.activation(sig[:], sp_ps[:], AF.Sigmoid,
                             scale=float(ws), bias=float(bs))
        rcp = ps_rec.tile([128, 2, 512], F32, tag="rc")
        nc.tensor.matmul(rcp[:, 0], MT[:, 0:128],
                         G[:, pt * 512:(pt + 1) * 512], start=True, stop=True)
        nc.tensor.matmul(rcp[:, 1], MT[:, 128:256],
                         G[:, pt * 512:(pt + 1) * 512], start=True, stop=True)
        R = ew.tile([128, 2, 512], BF16, tag="R")
        E = ew.tile([128, 2, 512], BF16, tag="E")
        for ct in range(2):
            nc.scalar.activation(R[:, ct], rcp[:, ct], AF.Identity,
                                 scale=spectral[:, ct:ct + 1],
                                 bias=brs[:, ct:ct + 1])
            nc.vector.scalar_tensor_tensor(
                E[:, ct], Dall[:, ct, pt], spectral[:, ct:ct + 1], sig[:],
                op0=ALU.mult, op1=ALU.mult)
        fu = outr.tile([128, 2, 512], BF16, tag="fu")
        nc.vector.tensor_tensor(fu[:], E[:], Qall[:, :, pt], op=ALU.add)
        E2 = ew.tile([128, 2, 512], BF16, tag="E2")
        nc.vector.tensor_tensor(
            E2[:], R[:], sig[:][:, None, :].broadcast_to([128, 2, 512]),
            op=ALU.mult)
        cp = outr.tile([128, 2, 512], BF16, tag="cp")
        nc.gpsimd.tensor_tensor(cp[:], E2[:], fm_sb[:, :, pt], op=ALU.add)
        for ct in range(2):
            nc.sync.dma_start(
                fo_v[:, ct, pt * 8:(pt + 1) * 8, :],
                fu[:, ct].rearrange("p (h w) -> p h w", h=8))
            nc.sync.dma_start(
                co_v[:, ct, pt * 8:(pt + 1) * 8, :],
                cp[:, ct].rearrange("p (h w) -> p h w", h=8))
    ctx.close()


def _prep_weights(W3, b3, Wa1, ba1, Wa2, ba2, Wa3, ba3, Wu, bu, Wr, br,
                  Wsa, bsa, Wsm, bsm):
    f = np.float32
    bf = ml_dtypes.bfloat16
    # w3t[p, kt, t, co] = W3[co, kt*128+p, dy, dx]
    w3t = np.ascontiguousarray(
        W3.reshape(C, 4, 128, 9).transpose(2, 1, 3, 0)).astype(bf)
    b3h = np.ascontiguousarray(b3.reshape(2, 128).T).astype(f)
    # wum[0, m, s, r] = (Wu @ Wa_m)[r, s]; bum[r, m] = (Wu @ ba_m + bu)[r]
    wum = np.stack([(Wu @ Wa1).T, (Wu @ Wa2).T, (Wu @ Wa3).T], axis=0)
    wum = np.ascontiguousarray(wum[None]).astype(f)
    bum = np.stack([Wu @ ba1 + bu, Wu @ ba2 + bu, Wu @ ba3 + bu], axis=1)
    bum = np.ascontiguousarray(bum).astype(f)
    wuwa1 = Wu @ Wa1
    wub1 = np.concatenate([wuwa1[:, 0], wuwa1[:, 1],
                           Wu @ ba1 + bu]).reshape(1, 12).astype(f)
    # wrt[p, kk, mm, m] = Wr[mm*128+m, kk*128+p]
    wrt = np.ascontiguousarray(
        Wr.reshape(2, 128, 2, 128).transpose(3, 2, 0, 1)).astype(f)
    brn = np.ascontiguousarray(br.reshape(2, 128).T).astype(f)
    # wsc[p, kk, mm, m]: kk<2 -> Wsa/128 (mean folded), kk>=2 -> Wsm
    wsa_r = (Wsa / 128.0).reshape(2, 128, 2, 128).transpose(3, 2, 0, 1)
    wsm_r = Wsm.reshape(2, 128, 2, 128).transpose(3, 2, 0, 1)
    wsc = np.ascontiguousarray(
        np.concatenate([wsa_r, wsm_r], axis=1)).astype(f)
    bsc = np.ascontiguousarray((bsa + bsm).reshape(2, 128).T).astype(f)
    return dict(w3t=w3t, b3=b3h, wum=wum, bum=bum, wub1=wub1, wrt=wrt,
                brn=brn, wsc=wsc, bsc=bsc)


_CACHE = {}


def kernel(frm_feat, other_feat, W3, b3, Wa1, ba1, Wa2, ba2, Wa3, ba3,
           Wu, bu, Wr, br, ws, bs, Wsa, bsa, Wsm, bsm, alpha, lam,
           _trace=False, _tmpdir=None):
    frm_feat = np.asarray(frm_feat, np.float32)
    other_feat = np.asarray(other_feat, np.float32)
    key = (float(alpha), float(ws), float(bs))
    if key not in _CACHE:
        _CACHE[key] = build_program(float(alpha), float(ws), float(bs))
    nc = _CACHE[key]

    wd = _prep_weights(np.asarray(W3), np.asarray(b3), np.asarray(Wa1),
                       np.asarray(ba1), np.asarray(Wa2), np.asarray(ba2),
                       np.asarray(Wa3), np.asarray(ba3), np.asarray(Wu),
                       np.asarray(bu), np.asarray(Wr), np.asarray(br),
                       np.asarray(Wsa), np.asarray(bsa), np.asarray(Wsm),
                       np.asarray(bsm))
    wd["lam"] = np.asarray(lam, np.float32).reshape(4, 1)

    in_maps = []
    for b_i in range(NCORES):
        m = dict(wd)
        m["frm"] = np.ascontiguousarray(frm_feat[b_i])
        m["oth"] = np.ascontiguousarray(other_feat[b_i])
        in_maps.append(m)

    res = bass_utils.run_bass_kernel_spmd(
        nc, in_maps, core_ids=list(range(NCORES)), trace=_trace,
        tmpdir=_tmpdir)
    fused = np.stack([np.asarray(res.results[i]["fused"])
                      for i in range(NCORES)]).astype(np.float32)
    cpr = np.stack([np.asarray(res.results[i]["cpr"])
                    for i in range(NCORES)]).astype(np.float32)
    kernel._last_exec_time_ns = res.exec_time_ns
    kernel._last_results = res
    return fused, cpr


# revision 36
# speedup vs baseline: 1.1380x; 1.1380x over previous
"""Trainium2 Bass kernel for nn_MDRMWithCPRecon.

Sharding: pure data parallel over batch B=8 -> one batch element per
NeuronCore (8 cores). All parameters replicated.

v2 changes vs baseline (294us):
  - bf16 conv (FWL weight loads, 216ns/MM vs fp32r 239ns), bf16
    everywhere downstream (DVE 2x elementwise, halved output DMA).
  - input staged in 16 quarter-chunks, converted f32->bf16 on
    scalar/vector engines, conv starts after first chunk (~7us vs 35us).
  - Fm kept in SBUF (kills the 8MB DRAM round trip).
  - U_gen folded host-side: u = (Wu@Wa) @ [avg;max] + (Wu@ba+bu) -> the
    whole adapter stage disappears.
  - recon bias br folded as a 5th row of G / MT.
  - final stage: ct-pairs processed in single wide ops, spread across
    scalar (Q, R, sig*spectral), vector (D, E, fu, E2) and gpsimd (cp).
"""

import numpy as np
import ml_dtypes

import concourse.bacc as bacc
import concourse.bass as bass
import concourse.tile as tile
from concourse import mybir, bass_utils

F32 = mybir.dt.float32
BF16 = mybir.dt.bfloat16
AF = mybir.ActivationFunctionType
ALU = mybir.AluOpType
AX = mybir.AxisListType

B, C, H, W, K = 8, 256, 64, 64, 4
HW = H * W
NCORES = 8


def build_program(alpha, ws, bs):
    from concourse.masks import make_identity

    nc = bacc.Bacc("TRN2", target_bir_lowering=False, debug=False,
                   num_devices=NCORES)

    frm = nc.dram_tensor("frm", [C, H, W], F32, kind="ExternalInput")
    oth = nc.dram_tensor("oth", [C, H, W], F32, kind="ExternalInput")
    w3t_d = nc.dram_tensor("w3t", [128, 4, 9, 256], BF16, kind="ExternalInput")
    b3_d = nc.dram_tensor("b3", [128, 2], F32, kind="ExternalInput")
    wum_d = nc.dram_tensor("wum", [1, 3, 2, 4], F32, kind="ExternalInput")
    bum_d = nc.dram_tensor("bum", [4, 3], F32, kind="ExternalInput")
    wub1_d = nc.dram_tensor("wub1", [1, 12], F32, kind="ExternalInput")
    wrt_d = nc.dram_tensor("wrt", [128, 2, 2, 128], F32, kind="ExternalInput")
    brn_d = nc.dram_tensor("brn", [128, 2], F32, kind="ExternalInput")
    wsc_d = nc.dram_tensor("wsc", [128, 4, 2, 128], F32, kind="ExternalInput")
    bsc_d = nc.dram_tensor("bsc", [128, 2], F32, kind="ExternalInput")
    lam_d = nc.dram_tensor("lam", [4, 1], F32, kind="ExternalInput")
    fused_o = nc.dram_tensor("fused", [C, H, W], BF16, kind="ExternalOutput")
    cpr_o = nc.dram_tensor("cpr", [C, H, W], BF16, kind="ExternalOutput")

    with tile.TileContext(nc) as tc:
        _build_tile(tc, nc, make_identity, locals(), alpha, ws, bs)
    nc.compile()
    return nc


def _build_tile(tc, nc, make_identity, T, alpha, ws, bs):
    frm, oth = T["frm"], T["oth"]
    w3t_d, b3_d, wum_d, bum_d = T["w3t_d"], T["b3_d"], T["wum_d"], T["bum_d"]
    wub1_d = T["wub1_d"]
    wrt_d, brn_d, wsc_d, bsc_d = (T["wrt_d"], T["brn_d"], T["wsc_d"],
                                  T["bsc_d"])
    lam_d, fused_o, cpr_o = T["lam_d"], T["fused_o"], T["cpr_o"]

    import contextlib
    ctx = contextlib.ExitStack()
    consts = ctx.enter_context(tc.tile_pool(name="consts", bufs=1))
    stage = ctx.enter_context(tc.tile_pool(name="stage", bufs=4))
    ew = ctx.enter_context(tc.tile_pool(name="ew", bufs=2))
    outr = ctx.enter_context(tc.tile_pool(name="outr", bufs=2))
    ps_conv = ctx.enter_context(tc.tile_pool(name="ps_conv", bufs=2, space="PSUM"))
    ps_sm = ctx.enter_context(tc.tile_pool(name="ps_sm", bufs=2, space="PSUM"))
    ps_spat = ctx.enter_context(tc.tile_pool(name="ps_spat", bufs=2, space="PSUM"))
    ps_rec = ctx.enter_context(tc.tile_pool(name="ps_rec", bufs=1, space="PSUM"))

    # ---- conv weights first on the scalar queue (kt0 gates first matmul) --
    w3_sb = consts.tile([128, 4, 9, 256], BF16)
    for kt in range(4):
        nc.scalar.dma_start(w3_sb[:, kt], w3t_d[:, kt])

    # ---- small weights to SBUF (sync queue) ----
    b3_sb = consts.tile([128, 2], F32)
    nc.sync.dma_start(b3_sb[:], b3_d[:])
    wum_sb = consts.tile([1, 3, 2, 4], F32)
    nc.sync.dma_start(wum_sb[:], wum_d[:])
    bum_sb = consts.tile([4, 3], F32)
    nc.sync.dma_start(bum_sb[:], bum_d[:])
    wub1_sb = consts.tile([1, 12], F32)
    nc.sync.dma_start(wub1_sb[:], wub1_d[:])
    wrt_sb = consts.tile([128, 2, 2, 128], F32)
    nc.sync.dma_start(wrt_sb[:], wrt_d[:])
    wsc_sb = consts.tile([128, 4, 2, 128], F32)
    nc.sync.dma_start(wsc_sb[:], wsc_d[:])
    bsc_sb = consts.tile([128, 2], F32)
    nc.sync.dma_start(bsc_sb[:], bsc_d[:])
    lam_sb = consts.tile([4, 1], F32)
    nc.sync.dma_start(lam_sb[:], lam_d[:])

    MT = consts.tile([4, 256], BF16)
    brn_sb = consts.tile([128, 2], F32)
    nc.sync.dma_start(brn_sb[:], brn_d[:])

    ident = consts.tile([128, 128], F32)
    make_identity(nc, ident[:])
    ones128 = consts.tile([128, 1], F32)
    nc.gpsimd.memset(ones128[:], 1.0)
    ones4 = consts.tile([4, 128], BF16)
    nc.gpsimd.memset(ones4[:], 1.0)
    ones_row = consts.tile([4, 128], F32)
    nc.gpsimd.memset(ones_row[:], 1.0)
    G = consts.tile([4, HW], BF16)          # CP factor outer products

    # gpsimd tensor_tensor ucode warm-up (IRAM load off the critical path)
    scrap3 = consts.tile([4, 8], BF16)
    nc.gpsimd.tensor_tensor(scrap3[:], ones4[:, 0:8], ones4[:, 8:16],
                            op=ALU.add)

    # ---- padded bf16 image [128, 4(kt), 66, 66]; kt 0/1 frm, 2/3 oth ----
    xr = consts.tile([128, 4, 66, 66], BF16)
    for kt in range(4):
        nc.vector.memset(xr[:, kt, 0:1, :], 0.0)
        nc.vector.memset(xr[:, kt, 65:66, :], 0.0)
        nc.vector.memset(xr[:, kt, 1:65, 0:1], 0.0)
        nc.vector.memset(xr[:, kt, 1:65, 65:66], 0.0)

    # stream input in 16 quarter-chunks over the 3 DMA-capable queues
    # (sync/gpsimd early; scalar queue is busy with w3). Convert f32->bf16
    # on DVE early (idle then) and ACT late (DVE is stats-busy by then).
    n_cv = 0
    for q in range(4):
        for kt in range(4):
            src = frm if kt < 2 else oth
            kt2 = kt % 2
            src_v = src.rearrange("(k p) h w -> p k h w", p=128)
            stg = stage.tile([128, 16, 64], F32, tag="stg")
            if q < 2:
                eng = nc.sync if n_cv % 2 == 0 else nc.gpsimd
            else:
                eng = (nc.sync, nc.gpsimd, nc.scalar)[n_cv % 3]
            eng.dma_start(stg[:], src_v[:, kt2, q * 16:(q + 1) * 16, :])
            dst = xr[:, kt, 1 + q * 16: 1 + (q + 1) * 16, 1:65]
            if (q < 2 and kt < 3):
                nc.vector.tensor_copy(dst, stg[:])
            else:
                nc.scalar.copy(dst, stg[:])
            n_cv += 1

    # preload ACT tables now (runs during conv, before the U-chain needs
    # Exp/Sigmoid/Identity)
    scrap = consts.tile([1, 8], F32)
    nc.vector.memset(scrap[:], 0.0)
    scrap2 = consts.tile([1, 8], F32)
    nc.scalar.activation(scrap2[0:1, 2:4], scrap[0:1, 2:4], AF.Exp)
    nc.scalar.activation(scrap2[0:1, 4:6], scrap[0:1, 4:6], AF.Sigmoid)
    nc.scalar.activation(scrap2[0:1, 6:8], scrap[0:1, 6:8], AF.Identity)

    # ---- pooled-stat tiles ----
    sums1 = consts.tile([128, 2, 8], F32)       # per-(ct,pt) sums of Fm
    q_sum = consts.tile([128, 2, 64], F32)      # sum over w  -> [c, h]
    q_max = consts.tile([128, 2, 64], F32)
    # ping-pong accumulators for the over-h stats (combined during conv)
    ppA = consts.tile([128, 2, 2, 64], F32)     # [sum/max, ct, w] bank A
    ppB = consts.tile([128, 2, 2, 64], F32)
    # pt-major so [:, pt] slices are contiguous (keeps DVE 2x bf16 mode)
    fm_sb = consts.tile([128, 8, 2, 512], BF16)  # Fm resident in SBUF
    Qall = consts.tile([128, 8, 2, 512], BF16)   # (1-a)*oth per tile
    Dall = consts.tile([128, 8, 2, 512], BF16)   # a*frm - Q per tile

    a = float(alpha)

    # ---- conv3x3 + leaky relu + streaming stats + Q/D precompute ----
    for pt in range(8):
        for ct in range(2):
            ps = ps_conv.tile([128, 512], F32, tag="conv")
            idx = 0
            for kt in range(4):
                for t in range(9):
                    dy, dx = t // 3, t % 3
                    nc.tensor.matmul(
                        ps[:],
                        w3_sb[:, kt, t, ct * 128:(ct + 1) * 128],
                        xr[:, kt, pt * 8 + dy: pt * 8 + dy + 8, dx: dx + 64],
                        start=(idx == 0), stop=(idx == 35))
                    idx += 1
            nc.scalar.activation(fm_sb[:, pt, ct], ps[:], AF.Lrelu,
                                 bias=b3_sb[:, ct:ct + 1], alpha=0.01,
                                 accum_out=sums1[:, ct, pt:pt + 1])
            blk = fm_sb[:, pt, ct].rearrange("p (h w) -> p h w", h=8)
            blk_t = fm_sb[:, pt, ct].rearrange("p (h w) -> p w h", h=8)
            nc.vector.tensor_reduce(q_sum[:, ct, pt * 8:(pt + 1) * 8], blk,
                                    axis=AX.X, op=ALU.add)
            nc.vector.tensor_reduce(q_max[:, ct, pt * 8:(pt + 1) * 8], blk,
                                    axis=AX.X, op=ALU.max)
            src_pp, dst_pp = (ppA, ppB) if pt % 2 else (ppB, ppA)
            if pt == 0:
                nc.vector.tensor_reduce(ppA[:, 0, ct, :], blk_t,
                                        axis=AX.X, op=ALU.add)
                nc.vector.tensor_reduce(ppA[:, 1, ct, :], blk_t,
                                        axis=AX.X, op=ALU.max)
            else:
                ppt = ew.tile([128, 2, 64], F32, tag="ppt")
                nc.vector.tensor_reduce(ppt[:, 0, :], blk_t,
                                        axis=AX.X, op=ALU.add)
                nc.vector.tensor_reduce(ppt[:, 1, :], blk_t,
                                        axis=AX.X, op=ALU.max)
                nc.vector.tensor_tensor(dst_pp[:, 0, ct, :],
                                        src_pp[:, 0, ct, :], ppt[:, 0, :],
                                        op=ALU.add)
                nc.vector.tensor_tensor(dst_pp[:, 1, ct, :],
                                        src_pp[:, 1, ct, :], ppt[:, 1, :],
                                        op=ALU.max)
            # Q/D for the final stage (only need xr; hide under conv)
            oth_t = xr[:, 2 + ct, 1 + pt * 8: 9 + pt * 8, 1:65]
            frm_t = xr[:, ct, 1 + pt * 8: 9 + pt * 8, 1:65]
            nc.scalar.activation(
                Qall[:, pt, ct].rearrange("p (h w) -> p h w", h=8), oth_t,
                AF.Copy, scale=float(1.0 - a))
            nc.vector.scalar_tensor_tensor(
                Dall[:, pt, ct].rearrange("p (h w) -> p h w", h=8), frm_t, a,
                Qall[:, pt, ct].rearrange("p (h w) -> p h w", h=8),
                op0=ALU.mult, op1=ALU.subtract)

    # ---- combine remaining partials (pp ended in ppB after 7 adds) ----
    p_sum = ppB[:, 0]       # [128, 2, 64]
    p_max = ppB[:, 1]
    sm1 = consts.tile([128, 4], F32)    # [sum1 ct0, ct1, max1 ct0, ct1]
    nc.vector.tensor_reduce(sm1[:, 0:2], sums1[:], axis=AX.X, op=ALU.add)
    nc.vector.tensor_reduce(sm1[:, 2:4], q_max[:], axis=AX.X, op=ALU.max)

    # ---- pooled rows (modes 2/3): rows_sb[0, s, :] = [m2(64) | m3(64)] ----
    rows_sb = consts.tile([1, 2, 128], F32)

    def colrows(S_sum, S_max, denom, off):
        ssum = ps_sm.tile([1, 64], F32, tag="sm")
        nc.tensor.matmul(ssum[:], ones128[:], S_sum[:, 0, :], start=True,
                         stop=False)
        nc.tensor.matmul(ssum[:], ones128[:], S_sum[:, 1, :], start=False,
                         stop=True)
        nc.scalar.mul(rows_sb[0:1, 0, off:off + 64], ssum[:], 1.0 / denom)
        tpm = ps_sm.tile([128, 128], F32, tag="sm")
        nc.tensor.transpose(tpm[:], S_max[:].rearrange("p c w -> p (c w)"),
                            ident[:])
        mxr = ew.tile([128, 1], F32, tag="mxr")
        nc.vector.tensor_reduce(mxr[:], tpm[:], axis=AX.X, op=ALU.max)
        tpb = ps_sm.tile([1, 128], F32, tag="sm")
        nc.tensor.transpose(tpb[:], mxr[:], ident[:])
        nc.vector.tensor_reduce(
            rows_sb[0:1, 1, off:off + 64],
            tpb[:].rearrange("p (c w) -> p w c", c=2), axis=AX.X, op=ALU.max)

    colrows(p_sum, p_max, C * H, 0)
    colrows(q_sum, q_max, C * W, 64)

    U1T = consts.tile([4, 256], BF16)
    uct = consts.tile([4, 128], BF16)    # [U2T(64) | U3T(64)]
    U1n = consts.tile([128, 2, 4], F32)

    def softmax4(src, cw, dst_n):
        """src [cw, 4] logits -> dst_n [cw, 4] softmax; returns dst_n."""
        negm = ew.tile([cw, 1], F32, tag="negm")
        nc.vector.tensor_reduce(negm[:], src, axis=AX.X, op=ALU.max,
                                negate=True)
        ssum = ew.tile([cw, 1], F32, tag="ssum")
        etile = ew.tile([cw, 4], F32, tag="etile")
        nc.scalar.activation(etile[:], src, AF.Exp, bias=negm[:],
                             accum_out=ssum[:])
        rec = ew.tile([cw, 1], F32, tag="rec")
        nc.vector.reciprocal(rec[:], ssum[:])
        nc.vector.tensor_scalar(dst_n, etile[:], rec[:], None, op0=ALU.mult)
        return dst_n

    # mode 1: logits computed directly channel-on-partition:
    # u1t[c, r] = wb0[r]*avg1[c] + wb1[r]*max1[c] + bum1[r]
    bc_ps = ps_sm.tile([128, 12], F32, tag="sm")
    nc.tensor.matmul(bc_ps[:], ones_row[0:1, :], wub1_sb[:], start=True,
                     stop=True)
    bc_sb = consts.tile([128, 12], F32)
    nc.scalar.mul(bc_sb[:], bc_ps[:], 1.0)
    for ct in range(2):
        s1 = ew.tile([128, 4], F32, tag="s1")
        nc.vector.scalar_tensor_tensor(s1[:], bc_sb[:, 0:4],
                                       sm1[:, ct:ct + 1], bc_sb[:, 8:12],
                                       op0=ALU.mult, op1=ALU.add)
        u1t = ew.tile([128, 4], F32, tag="u1t")
        nc.vector.scalar_tensor_tensor(u1t[:], bc_sb[:, 4:8],
                                       sm1[:, 2 + ct:3 + ct], s1[:],
                                       op0=ALU.mult, op1=ALU.add)
        softmax4(u1t[:], 128, U1n[:, ct, :])
        tb_ps = ps_sm.tile([4, 128], F32, tag="sm")
        nc.tensor.transpose(tb_ps[:], U1n[:, ct, :], ident[:])
        nc.scalar.copy(U1T[:, ct * 128:(ct + 1) * 128], tb_ps[:])

    # modes 2/3: rank-1 matmuls from pooled rows, transpose, softmax
    for m, off in ((1, 0), (2, 64)):
        u_ps = ps_sm.tile([4, 64], F32, tag="sm")
        nc.tensor.matmul(u_ps[:], wum_sb[0:1, m, 0, :],
                         rows_sb[0:1, 0, off:off + 64], start=True, stop=False)
        nc.tensor.matmul(u_ps[:], wum_sb[0:1, m, 1, :],
                         rows_sb[0:1, 1, off:off + 64], start=False, stop=True)
        u_sb = consts.tile([4, 64], F32, tag=f"u{m}")
        nc.scalar.activation(u_sb[:], u_ps[:], AF.Identity,
                             bias=bum_sb[:, m:m + 1])
        ut_ps = ps_sm.tile([64, 4], F32, tag="sm")
        nc.tensor.transpose(ut_ps[:], u_sb[:], ident[0:4, 0:4])
        sm_t = ew.tile([64, 4], F32, tag="smt")
        softmax4(ut_ps[:], 64, sm_t[:])
        tb_ps = ps_sm.tile([4, 64], F32, tag="sm")
        nc.tensor.transpose(tb_ps[:], sm_t[:], ident[0:64, 0:64])
        nc.scalar.copy(uct[:, (m - 1) * 64:m * 64], tb_ps[:])

    # ---- G: G[r, h, w] = U3T[r,h]*U2T[r,w] (DVE/GPS halves) ----
    Gv = G[:].rearrange("p (h w) -> p h w", h=64)
    nc.vector.tensor_tensor(
        Gv[:, 0:32, :],
        uct[0:4, 64:96][:, :, None].broadcast_to([4, 32, 64]),
        uct[0:4, 0:64][:, None, :].broadcast_to([4, 32, 64]), op=ALU.mult)
    nc.gpsimd.tensor_tensor(
        Gv[:, 32:64, :],
        uct[0:4, 96:128][:, :, None].broadcast_to([4, 32, 64]),
        uct[0:4, 0:64][:, None, :].broadcast_to([4, 32, 64]), op=ALU.mult)

    # ---- MT rows 0-3 = (Wr @ U1 diag(lam))^T ----
    for mm in range(2):
        m_ps = ps_sm.tile([128, 4], F32, tag="sm")
        for kk in range(2):
            nc.tensor.matmul(m_ps[:], wrt_sb[:, kk, mm, :], U1n[:, kk, :],
                             start=(kk == 0), stop=(kk == 1))
        m_sb = ew.tile([128, 4], F32, tag="msb")
        nc.scalar.copy(m_sb[:], m_ps[:])
        mt_ps = ps_sm.tile([4, 128], F32, tag="sm")
        nc.tensor.transpose(mt_ps[:], m_sb[:], ident[:])
        nc.vector.tensor_scalar(MT[:, mm * 128:(mm + 1) * 128], mt_ps[:],
                                lam_sb[:], None, op0=ALU.mult)

    # ---- spectral attention ----
    gag = consts.tile([128, 4], F32)  # [ga_ct0, ga_ct1, gm_ct0, gm_ct1]
    for ct in range(2):
        f_ps = ps_sm.tile([128, 128], F32, tag="sm")
        nc.tensor.matmul(f_ps[:], U1T[:, ct * 128:(ct + 1) * 128], uct[:],
                         start=True, stop=True)
        nc.vector.tensor_reduce(gag[:, ct:ct + 1], f_ps[:], axis=AX.X,
                                op=ALU.add)
        nc.vector.tensor_reduce(gag[:, 2 + ct:3 + ct], f_ps[:], axis=AX.X,
                                op=ALU.max)
    spectral = consts.tile([128, 2], F32)
    for mm in range(2):
        sp_ps = ps_sm.tile([128, 1], F32, tag="sm")
        for kk in range(4):
            nc.tensor.matmul(sp_ps[:], wsc_sb[:, kk, mm, :],
                             gag[:, kk:kk + 1], start=(kk == 0),
                             stop=(kk == 3))
        stmp = ew.tile([128, 1], F32, tag="stmp")
        nc.scalar.activation(stmp[:], sp_ps[:], AF.Sigmoid,
                             bias=bsc_sb[:, mm:mm + 1])
        nc.scalar.activation(spectral[:, mm:mm + 1], stmp[:], AF.Sigmoid)

    # ---- final elementwise stage ----
    # fused = D*sigc + Q;  cp_recon = (rc + br)*sigc + Fm; sigc = spc*sig
    fo_v = fused_o.rearrange("(c p) h w -> p c h w", p=128)
    co_v = cpr_o.rearrange("(c p) h w -> p c h w", p=128)
    for pt in range(8):
        sp_ps = ps_spat.tile([128, 512], F32, tag="spat")
        nc.tensor.matmul(sp_ps[:], ones4[:], G[:, pt * 512:(pt + 1) * 512],
                         start=True, stop=True)
        sig = ew.tile([128, 512], BF16, tag="sig")
        nc.scalar.activation(sig[:], sp_ps[:], AF.Sigmoid,
                             scale=float(ws), bias=float(bs))
        sigc = ew.tile([128, 2, 512], BF16, tag="sigc")
        for ct in range(2):
            nc.vector.tensor_scalar(sigc[:, ct], sig[:],
                                    spectral[:, ct:ct + 1], None,
                                    op0=ALU.mult)
        rcp = ps_rec.tile([128, 2, 512], F32, tag="rc")
        nc.tensor.matmul(rcp[:, 0], MT[:, 0:128],
                         G[:, pt * 512:(pt + 1) * 512], start=True, stop=True)
        nc.tensor.matmul(rcp[:, 1], MT[:, 128:256],
                         G[:, pt * 512:(pt + 1) * 512], start=True, stop=True)
        R = ew.tile([128, 2, 512], BF16, tag="R")
        for ct in range(2):
            nc.scalar.activation(R[:, ct], rcp[:, ct], AF.Identity,
                                 bias=brn_sb[:, ct:ct + 1])
        E = ew.tile([128, 2, 512], BF16, tag="E")
        nc.vector.tensor_tensor(E[:], Dall[:, pt], sigc[:], op=ALU.mult)
        fu = outr.tile([128, 2, 512], BF16, tag="fu")
        nc.vector.tensor_tensor(fu[:], E[:], Qall[:, pt], op=ALU.add)
        E2 = ew.tile([128, 2, 512], BF16, tag="E2")
        nc.vector.tensor_tensor(E2[:], R[:], sigc[:], op=ALU.mult)
        cp = outr.tile([128, 2, 512], BF16, tag="cp")
        nc.gpsimd.tensor_tensor(cp[:], E2[:], fm_sb[:, pt], op=ALU.add)
        for ct in range(2):
            nc.sync.dma_start(
                fo_v[:, ct, pt * 8:(pt + 1) * 8, :],
                fu[:, ct].rearrange("p (h w) -> p h w", h=8))
            nc.sync.dma_start(
                co_v[:, ct, pt * 8:(pt + 1) * 8, :],
                cp[:, ct].rearrange("p (h w) -> p h w", h=8))
    ctx.close()


def _prep_weights(W3, b3, Wa1, ba1, Wa2, ba2, Wa3, ba3, Wu, bu, Wr, br,
                  Wsa, bsa, Wsm, bsm):
    f = np.float32
    bf = ml_dtypes.bfloat16
    # w3t[p, kt, t, co] = W3[co, kt*128+p, dy, dx]
    w3t = np.ascontiguousarray(
        W3.reshape(C, 4, 128, 9).transpose(2, 1, 3, 0)).astype(bf)
    b3h = np.ascontiguousarray(b3.reshape(2, 128).T).astype(f)
    # wum[0, m, s, r] = (Wu @ Wa_m)[r, s]; bum[r, m] = (Wu @ ba_m + bu)[r]
    wum = np.stack([(Wu @ Wa1).T, (Wu @ Wa2).T, (Wu @ Wa3).T], axis=0)
    wum = np.ascontiguousarray(wum[None]).astype(f)
    bum = np.stack([Wu @ ba1 + bu, Wu @ ba2 + bu, Wu @ ba3 + bu], axis=1)
    bum = np.ascontiguousarray(bum).astype(f)
    wuwa1 = Wu @ Wa1
    wub1 = np.concatenate([wuwa1[:, 0], wuwa1[:, 1],
                           Wu @ ba1 + bu]).reshape(1, 12).astype(f)
    # wrt[p, kk, mm, m] = Wr[mm*128+m, kk*128+p]
    wrt = np.ascontiguousarray(
        Wr.reshape(2, 128, 2, 128).transpose(3, 2, 0, 1)).astype(f)
    brn = np.ascontiguousarray(br.reshape(2, 128).T).astype(f)
    # wsc[p, kk, mm, m]: kk<2 -> Wsa/128 (mean folded), kk>=2 -> Wsm
    wsa_r = (Wsa / 128.0).reshape(2, 128, 2, 128).transpose(3, 2, 0, 1)
    wsm_r = Wsm.reshape(2, 128, 2, 128).transpose(3, 2, 0, 1)
    wsc = np.ascontiguousarray(
        np.concatenate([wsa_r, wsm_r], axis=1)).astype(f)
    bsc = np.ascontiguousarray((bsa + bsm).reshape(2, 128).T).astype(f)
    return dict(w3t=w3t, b3=b3h, wum=wum, bum=bum, wub1=wub1, wrt=wrt,
                brn=brn, wsc=wsc, bsc=bsc)


_CACHE = {}


def kernel(frm_feat, other_feat, W3, b3, Wa1, ba1, Wa2, ba2, Wa3, ba3,
           Wu, bu, Wr, br, ws, bs, Wsa, bsa, Wsm, bsm, alpha, lam,
           _trace=False, _tmpdir=None):
    frm_feat = np.asarray(frm_feat, np.float32)
    other_feat = np.asarray(other_feat, np.float32)
    key = (float(alpha), float(ws), float(bs))
    if key not in _CACHE:
        _CACHE[key] = build_program(float(alpha), float(ws), float(bs))
    nc = _CACHE[key]

    wd = _prep_weights(np.asarray(W3), np.asarray(b3), np.asarray(Wa1),
                       np.asarray(ba1), np.asarray(Wa2), np.asarray(ba2),
                       np.asarray(Wa3), np.asarray(ba3), np.asarray(Wu),
                       np.asarray(bu), np.asarray(Wr), np.asarray(br),
                       np.asarray(Wsa), np.asarray(bsa), np.asarray(Wsm),
                       np.asarray(bsm))
    wd["lam"] = np.asarray(lam, np.float32).reshape(4, 1)

    in_maps = []
    for b_i in range(NCORES):
        m = dict(wd)
        m["frm"] = np.ascontiguousarray(frm_feat[b_i])
        m["oth"] = np.ascontiguousarray(other_feat[b_i])
        in_maps.append(m)

    res = bass_utils.run_bass_kernel_spmd(
        nc, in_maps, core_ids=list(range(NCORES)), trace=_trace,
        tmpdir=_tmpdir)
    fused = np.stack([np.asarray(res.results[i]["fused"])
                      for i in range(NCORES)]).astype(np.float32)
    cpr = np.stack([np.asarray(res.results[i]["cpr"])
                    for i in range(NCORES)]).astype(np.float32)
    kernel._last_exec_time_ns = res.exec_time_ns
    kernel._last_results = res
    return fused, cpr


# revision 43
# speedup vs baseline: 1.1407x; 1.0023x over previous
"""Trainium2 Bass kernel for nn_MDRMWithCPRecon.

Sharding: pure data parallel over batch B=8 -> one batch element per
NeuronCore (8 cores). All parameters replicated.

v2 changes vs baseline (294us):
  - bf16 conv (FWL weight loads, 216ns/MM vs fp32r 239ns), bf16
    everywhere downstream (DVE 2x elementwise, halved output DMA).
  - input staged in 16 quarter-chunks, converted f32->bf16 on
    scalar/vector engines, conv starts after first chunk (~7us vs 35us).
  - Fm kept in SBUF (kills the 8MB DRAM round trip).
  - U_gen folded host-side: u = (Wu@Wa) @ [avg;max] + (Wu@ba+bu) -> the
    whole adapter stage disappears.
  - recon bias br folded as a 5th row of G / MT.
  - final stage: ct-pairs processed in single wide ops, spread across
    scalar (Q, R, sig*spectral), vector (D, E, fu, E2) and gpsimd (cp).
"""

import numpy as np
import ml_dtypes

import concourse.bacc as bacc
import concourse.bass as bass
import concourse.tile as tile
from concourse import mybir, bass_utils

F32 = mybir.dt.float32
BF16 = mybir.dt.bfloat16
AF = mybir.ActivationFunctionType
ALU = mybir.AluOpType
AX = mybir.AxisListType

B, C, H, W, K = 8, 256, 64, 64, 4
HW = H * W
NCORES = 8


def build_program(alpha, ws, bs):
    from concourse.masks import make_identity

    nc = bacc.Bacc("TRN2", target_bir_lowering=False, debug=False,
                   num_devices=NCORES)

    frm = nc.dram_tensor("frm", [C, H, W], F32, kind="ExternalInput")
    oth = nc.dram_tensor("oth", [C, H, W], F32, kind="ExternalInput")
    w3t_d = nc.dram_tensor("w3t", [128, 4, 9, 256], BF16, kind="ExternalInput")
    # all small weights packed into two tensors (one DMA each):
    # smallw[p, :] = [wrt(512) | wsc(1024) | b3(2) | brn(2) | bsc(2)]
    # tiny[r, :]   = [bum(3) | lam(1) | wum(24, row0) | wub1(12, row0)]
    smallw_d = nc.dram_tensor("smallw", [128, 1542], F32, kind="ExternalInput")
    tiny_d = nc.dram_tensor("tiny", [4, 40], F32, kind="ExternalInput")
    fused_o = nc.dram_tensor("fused", [C, H, W], BF16, kind="ExternalOutput")
    cpr_o = nc.dram_tensor("cpr", [C, H, W], BF16, kind="ExternalOutput")

    with tile.TileContext(nc) as tc:
        _build_tile(tc, nc, make_identity, locals(), alpha, ws, bs)
    nc.compile()
    return nc


def _build_tile(tc, nc, make_identity, T, alpha, ws, bs):
    frm, oth = T["frm"], T["oth"]
    w3t_d, smallw_d, tiny_d = T["w3t_d"], T["smallw_d"], T["tiny_d"]
    fused_o, cpr_o = T["fused_o"], T["cpr_o"]

    import contextlib
    ctx = contextlib.ExitStack()
    consts = ctx.enter_context(tc.tile_pool(name="consts", bufs=1))
    stage = ctx.enter_context(tc.tile_pool(name="stage", bufs=4))
    ew = ctx.enter_context(tc.tile_pool(name="ew", bufs=2))
    outr = ctx.enter_context(tc.tile_pool(name="outr", bufs=2))
    ps_conv = ctx.enter_context(tc.tile_pool(name="ps_conv", bufs=2, space="PSUM"))
    ps_sm = ctx.enter_context(tc.tile_pool(name="ps_sm", bufs=2, space="PSUM"))
    ps_spat = ctx.enter_context(tc.tile_pool(name="ps_spat", bufs=2, space="PSUM"))
    ps_rec = ctx.enter_context(tc.tile_pool(name="ps_rec", bufs=1, space="PSUM"))

    # ---- conv weights on the scalar queue, ct0 column-halves first so the
    # first conv tile's weights land ASAP ----
    w3_sb = consts.tile([128, 4, 9, 256], BF16)
    for ch in range(2):
        for kt in range(4):
            nc.scalar.dma_start(w3_sb[:, kt, :, ch * 128:(ch + 1) * 128],
                                w3t_d[:, kt, :, ch * 128:(ch + 1) * 128])

    # ---- packed small weights (issued later; not needed until U-chain) ---
    smallw_sb = consts.tile([128, 1542], F32)
    tiny_sb = consts.tile([4, 40], F32)
    wrt_sb = smallw_sb[:, 0:512].rearrange("p (kk mm m) -> p kk mm m", kk=2,
                                           mm=2)
    wsc_sb = smallw_sb[:, 512:1536].rearrange("p (kk mm m) -> p kk mm m",
                                              kk=4, mm=2)
    b3_sb = smallw_sb[:, 1536:1538]
    brn_sb = smallw_sb[:, 1538:1540]
    bsc_sb = smallw_sb[:, 1540:1542]
    bum_sb = tiny_sb[:, 0:3]
    lam_sb = tiny_sb[:, 3:4]
    wum_sb = tiny_sb[0:1, 4:28].rearrange("p (m s r) -> p m s r", m=3, s=2)
    wub1_sb = tiny_sb[0:1, 28:40]

    MT = consts.tile([4, 256], BF16)

    ident = consts.tile([128, 128], F32)
    make_identity(nc, ident[:])
    ones128 = consts.tile([128, 1], F32)
    nc.gpsimd.memset(ones128[:], 1.0)
    ones4 = consts.tile([4, 128], BF16)
    nc.gpsimd.memset(ones4[:], 1.0)
    ones_row = consts.tile([4, 128], F32)
    nc.gpsimd.memset(ones_row[:], 1.0)
    G = consts.tile([4, HW], BF16)          # CP factor outer products

    # gpsimd tensor_tensor ucode warm-up (IRAM load off the critical path)
    scrap3 = consts.tile([4, 8], BF16)
    nc.gpsimd.tensor_tensor(scrap3[:], ones4[:, 0:8], ones4[:, 8:16],
                            op=ALU.add)

    # ---- padded bf16 image [128, 4(kt), 66, 66]; kt 0/1 frm, 2/3 oth ----
    xr = consts.tile([128, 4, 66, 66], BF16)
    for kt in range(4):
        nc.vector.memset(xr[:, kt, 0:1, :], 0.0)
        nc.vector.memset(xr[:, kt, 65:66, :], 0.0)
        nc.vector.memset(xr[:, kt, 1:65, 0:1], 0.0)
        nc.vector.memset(xr[:, kt, 1:65, 65:66], 0.0)

    # stream input in 16 quarter-chunks over the 3 DMA-capable queues
    # (sync/gpsimd early; scalar queue is busy with w3). Convert f32->bf16
    # on DVE early (idle then) and ACT late (DVE is stats-busy by then).
    n_cv = 0
    for q in range(4):
        for kt in range(4):
            src = frm if kt < 2 else oth
            kt2 = kt % 2
            src_v = src.rearrange("(k p) h w -> p k h w", p=128)
            stg = stage.tile([128, 16, 64], F32, tag="stg")
            if q < 2:
                eng = nc.sync if n_cv % 2 == 0 else nc.gpsimd
            else:
                eng = (nc.sync, nc.gpsimd, nc.scalar)[n_cv % 3]
            eng.dma_start(stg[:], src_v[:, kt2, q * 16:(q + 1) * 16, :])
            dst = xr[:, kt, 1 + q * 16: 1 + (q + 1) * 16, 1:65]
            if q < 2:
                nc.vector.tensor_copy(dst, stg[:])
            else:
                nc.scalar.copy(dst, stg[:])
            n_cv += 1

    # packed small weights after the input chunks (sync queue is free then)
    nc.sync.dma_start(smallw_sb[:], smallw_d[:])
    nc.sync.dma_start(tiny_sb[:], tiny_d[:])

    # preload ACT tables now (runs during conv, before the U-chain needs
    # Exp/Sigmoid/Identity)
    scrap = consts.tile([1, 8], F32)
    nc.vector.memset(scrap[:], 0.0)
    scrap2 = consts.tile([1, 8], F32)
    nc.scalar.activation(scrap2[0:1, 2:4], scrap[0:1, 2:4], AF.Exp)
    nc.scalar.activation(scrap2[0:1, 4:6], scrap[0:1, 4:6], AF.Sigmoid)
    nc.scalar.activation(scrap2[0:1, 6:8], scrap[0:1, 6:8], AF.Identity)

    # ---- pooled-stat tiles ----
    sums1 = consts.tile([128, 2, 8], F32)       # per-(ct,pt) sums of Fm
    q_sum = consts.tile([128, 2, 64], F32)      # sum over w  -> [c, h]
    q_max = consts.tile([128, 2, 64], F32)
    # ping-pong accumulators for the over-h stats (combined during conv)
    ppA = consts.tile([128, 2, 2, 64], F32)     # [sum/max, ct, w] bank A
    ppB = consts.tile([128, 2, 2, 64], F32)
    # pt-major so [:, pt] slices are contiguous (keeps DVE 2x bf16 mode)
    fm_sb = consts.tile([128, 8, 2, 512], BF16)  # Fm resident in SBUF
    Qall = consts.tile([128, 8, 2, 512], BF16)   # (1-a)*oth per tile
    Dall = consts.tile([128, 8, 2, 512], BF16)   # a*frm - Q per tile

    a = float(alpha)

    # ---- conv3x3 + leaky relu + streaming stats + Q/D precompute ----
    for pt in range(8):
        for ct in range(2):
            ps = ps_conv.tile([128, 512], F32, tag="conv")
            idx = 0
            for kt in range(4):
                for t in range(9):
                    dy, dx = t // 3, t % 3
                    nc.tensor.matmul(
                        ps[:],
                        w3_sb[:, kt, t, ct * 128:(ct + 1) * 128],
                        xr[:, kt, pt * 8 + dy: pt * 8 + dy + 8, dx: dx + 64],
                        start=(idx == 0), stop=(idx == 35))
                    idx += 1
            nc.scalar.activation(fm_sb[:, pt, ct], ps[:], AF.Lrelu,
                                 bias=b3_sb[:, ct:ct + 1], alpha=0.01,
                                 accum_out=sums1[:, ct, pt:pt + 1])
            blk = fm_sb[:, pt, ct].rearrange("p (h w) -> p h w", h=8)
            blk_t = fm_sb[:, pt, ct].rearrange("p (h w) -> p w h", h=8)
            nc.vector.tensor_reduce(q_sum[:, ct, pt * 8:(pt + 1) * 8], blk,
                                    axis=AX.X, op=ALU.add)
            nc.vector.tensor_reduce(q_max[:, ct, pt * 8:(pt + 1) * 8], blk,
                                    axis=AX.X, op=ALU.max)
            src_pp, dst_pp = (ppA, ppB) if pt % 2 else (ppB, ppA)
            if pt == 0:
                nc.vector.tensor_reduce(ppA[:, 0, ct, :], blk_t,
                                        axis=AX.X, op=ALU.add)
                nc.vector.tensor_reduce(ppA[:, 1, ct, :], blk_t,
                                        axis=AX.X, op=ALU.max)
            else:
                ppt = ew.tile([128, 2, 64], F32, tag="ppt")
                nc.vector.tensor_reduce(ppt[:, 0, :], blk_t,
                                        axis=AX.X, op=ALU.add)
                nc.vector.tensor_reduce(ppt[:, 1, :], blk_t,
                                        axis=AX.X, op=ALU.max)
                nc.vector.tensor_tensor(dst_pp[:, 0, ct, :],
                                        src_pp[:, 0, ct, :], ppt[:, 0, :],
                                        op=ALU.add)
                nc.vector.tensor_tensor(dst_pp[:, 1, ct, :],
                                        src_pp[:, 1, ct, :], ppt[:, 1, :],
                                        op=ALU.max)
            # Q/D for the final stage (only need xr; hide under conv)
            oth_t = xr[:, 2 + ct, 1 + pt * 8: 9 + pt * 8, 1:65]
            frm_t = xr[:, ct, 1 + pt * 8: 9 + pt * 8, 1:65]
            nc.scalar.activation(
                Qall[:, pt, ct].rearrange("p (h w) -> p h w", h=8), oth_t,
                AF.Copy, scale=float(1.0 - a))
            nc.vector.scalar_tensor_tensor(
                Dall[:, pt, ct].rearrange("p (h w) -> p h w", h=8), frm_t, a,
                Qall[:, pt, ct].rearrange("p (h w) -> p h w", h=8),
                op0=ALU.mult, op1=ALU.subtract)

    # ---- combine remaining partials (pp ended in ppB after 7 adds) ----
    p_sum = ppB[:, 0]       # [128, 2, 64]
    p_max = ppB[:, 1]
    sm1 = consts.tile([128, 4], F32)    # [sum1 ct0, ct1, max1 ct0, ct1]
    nc.vector.tensor_reduce(sm1[:, 0:2], sums1[:], axis=AX.X, op=ALU.add)
    nc.vector.tensor_reduce(sm1[:, 2:4], q_max[:], axis=AX.X, op=ALU.max)

    # ---- pooled rows (modes 2/3): rows_sb[0, s, :] = [m2(64) | m3(64)] ----
    rows_sb = consts.tile([1, 2, 128], F32)

    def colrows(S_sum, S_max, denom, off):
        ssum = ps_sm.tile([1, 64], F32, tag="sm")
        nc.tensor.matmul(ssum[:], ones128[:], S_sum[:, 0, :], start=True,
                         stop=False)
        nc.tensor.matmul(ssum[:], ones128[:], S_sum[:, 1, :], start=False,
                         stop=True)
        nc.scalar.mul(rows_sb[0:1, 0, off:off + 64], ssum[:], 1.0 / denom)
        tpm = ps_sm.tile([128, 128], F32, tag="sm")
        nc.tensor.transpose(tpm[:], S_max[:].rearrange("p c w -> p (c w)"),
                            ident[:])
        mxr = ew.tile([128, 1], F32, tag="mxr")
        nc.vector.tensor_reduce(mxr[:], tpm[:], axis=AX.X, op=ALU.max)
        tpb = ps_sm.tile([1, 128], F32, tag="sm")
        nc.tensor.transpose(tpb[:], mxr[:], ident[:])
        nc.vector.tensor_reduce(
            rows_sb[0:1, 1, off:off + 64],
            tpb[:].rearrange("p (c w) -> p w c", c=2), axis=AX.X, op=ALU.max)

    colrows(p_sum, p_max, C * H, 0)
    colrows(q_sum, q_max, C * W, 64)

    U1T = consts.tile([4, 256], BF16)
    uct = consts.tile([4, 128], BF16)    # [U2T(64) | U3T(64)]
    U1n = consts.tile([128, 2, 4], F32)

    def softmax4(src, cw, dst_n):
        """src [cw, 4] logits -> dst_n [cw, 4] softmax; returns dst_n."""
        negm = ew.tile([cw, 1], F32, tag="negm")
        nc.vector.tensor_reduce(negm[:], src, axis=AX.X, op=ALU.max,
                                negate=True)
        ssum = ew.tile([cw, 1], F32, tag="ssum")
        etile = ew.tile([cw, 4], F32, tag="etile")
        nc.scalar.activation(etile[:], src, AF.Exp, bias=negm[:],
                             accum_out=ssum[:])
        rec = ew.tile([cw, 1], F32, tag="rec")
        nc.vector.reciprocal(rec[:], ssum[:])
        nc.vector.tensor_scalar(dst_n, etile[:], rec[:], None, op0=ALU.mult)
        return dst_n

    # mode 1: logits computed directly channel-on-partition:
    # u1t[c, r] = wb0[r]*avg1[c] + wb1[r]*max1[c] + bum1[r]
    bc_ps = ps_sm.tile([128, 12], F32, tag="sm")
    nc.tensor.matmul(bc_ps[:], ones_row[0:1, :], wub1_sb[:], start=True,
                     stop=True)
    bc_sb = consts.tile([128, 12], F32)
    nc.scalar.mul(bc_sb[:], bc_ps[:], 1.0)
    for ct in range(2):
        s1 = ew.tile([128, 4], F32, tag="s1")
        nc.vector.scalar_tensor_tensor(s1[:], bc_sb[:, 0:4],
                                       sm1[:, ct:ct + 1], bc_sb[:, 8:12],
                                       op0=ALU.mult, op1=ALU.add)
        u1t = ew.tile([128, 4], F32, tag="u1t")
        nc.vector.scalar_tensor_tensor(u1t[:], bc_sb[:, 4:8],
                                       sm1[:, 2 + ct:3 + ct], s1[:],
                                       op0=ALU.mult, op1=ALU.add)
        softmax4(u1t[:], 128, U1n[:, ct, :])
        tb_ps = ps_sm.tile([4, 128], F32, tag="sm")
        nc.tensor.transpose(tb_ps[:], U1n[:, ct, :], ident[:])
        nc.scalar.copy(U1T[:, ct * 128:(ct + 1) * 128], tb_ps[:])

    # modes 2/3: rank-1 matmuls from pooled rows, transpose, softmax
    for m, off in ((1, 0), (2, 64)):
        u_ps = ps_sm.tile([4, 64], F32, tag="sm")
        nc.tensor.matmul(u_ps[:], wum_sb[0:1, m, 0, :],
                         rows_sb[0:1, 0, off:off + 64], start=True, stop=False)
        nc.tensor.matmul(u_ps[:], wum_sb[0:1, m, 1, :],
                         rows_sb[0:1, 1, off:off + 64], start=False, stop=True)
        u_sb = consts.tile([4, 64], F32, tag=f"u{m}")
        nc.scalar.activation(u_sb[:], u_ps[:], AF.Identity,
                             bias=bum_sb[:, m:m + 1])
        ut_ps = ps_sm.tile([64, 4], F32, tag="sm")
        nc.tensor.transpose(ut_ps[:], u_sb[:], ident[0:4, 0:4])
        sm_t = ew.tile([64, 4], F32, tag="smt")
        softmax4(ut_ps[:], 64, sm_t[:])
        tb_ps = ps_sm.tile([4, 64], F32, tag="sm")
        nc.tensor.transpose(tb_ps[:], sm_t[:], ident[0:64, 0:64])
        nc.scalar.copy(uct[:, (m - 1) * 64:m * 64], tb_ps[:])

    # ---- G: G[r, h, w] = U3T[r,h]*U2T[r,w] (DVE/GPS halves) ----
    Gv = G[:].rearrange("p (h w) -> p h w", h=64)
    nc.vector.tensor_tensor(
        Gv[:, 0:32, :],
        uct[0:4, 64:96][:, :, None].broadcast_to([4, 32, 64]),
        uct[0:4, 0:64][:, None, :].broadcast_to([4, 32, 64]), op=ALU.mult)
    nc.gpsimd.tensor_tensor(
        Gv[:, 32:64, :],
        uct[0:4, 96:128][:, :, None].broadcast_to([4, 32, 64]),
        uct[0:4, 0:64][:, None, :].broadcast_to([4, 32, 64]), op=ALU.mult)

    # ---- MT rows 0-3 = (Wr @ U1 diag(lam))^T ----
    for mm in range(2):
        m_ps = ps_sm.tile([128, 4], F32, tag="sm")
        for kk in range(2):
            nc.tensor.matmul(m_ps[:], wrt_sb[:, kk, mm, :], U1n[:, kk, :],
                             start=(kk == 0), stop=(kk == 1))
        m_sb = ew.tile([128, 4], F32, tag="msb")
        nc.scalar.copy(m_sb[:], m_ps[:])
        mt_ps = ps_sm.tile([4, 128], F32, tag="sm")
        nc.tensor.transpose(mt_ps[:], m_sb[:], ident[:])
        nc.vector.tensor_scalar(MT[:, mm * 128:(mm + 1) * 128], mt_ps[:],
                                lam_sb[:], None, op0=ALU.mult)

    # ---- spectral attention ----
    gag = consts.tile([128, 4], F32)  # [ga_ct0, ga_ct1, gm_ct0, gm_ct1]
    for ct in range(2):
        f_ps = ps_sm.tile([128, 128], F32, tag="sm")
        nc.tensor.matmul(f_ps[:], U1T[:, ct * 128:(ct + 1) * 128], uct[:],
                         start=True, stop=True)
        nc.vector.tensor_reduce(gag[:, ct:ct + 1], f_ps[:], axis=AX.X,
                                op=ALU.add)
        nc.vector.tensor_reduce(gag[:, 2 + ct:3 + ct], f_ps[:], axis=AX.X,
                                op=ALU.max)
    spectral = consts.tile([128, 2], F32)
    for mm in range(2):
        sp_ps = ps_sm.tile([128, 1], F32, tag="sm")
        for kk in range(4):
            nc.tensor.matmul(sp_ps[:], wsc_sb[:, kk, mm, :],
                             gag[:, kk:kk + 1], start=(kk == 0),
                             stop=(kk == 3))
        stmp = ew.tile([128, 1], F32, tag="stmp")
        nc.scalar.activation(stmp[:], sp_ps[:], AF.Sigmoid,
                             bias=bsc_sb[:, mm:mm + 1])
        nc.scalar.activation(spectral[:, mm:mm + 1], stmp[:], AF.Sigmoid)

    # ---- final elementwise stage ----
    # fused = D*sigc + Q;  cp_recon = (rc + br)*sigc + Fm; sigc = spc*sig
    fo_v = fused_o.rearrange("(c p) h w -> p c h w", p=128)
    co_v = cpr_o.rearrange("(c p) h w -> p c h w", p=128)
    for pt in range(8):
        sp_ps = ps_spat.tile([128, 512], F32, tag="spat")
        nc.tensor.matmul(sp_ps[:], ones4[:], G[:, pt * 512:(pt + 1) * 512],
                         start=True, stop=True)
        sig = ew.tile([128, 512], BF16, tag="sig")
        nc.scalar.activation(sig[:], sp_ps[:], AF.Sigmoid,
                             scale=float(ws), bias=float(bs))
        sigc = ew.tile([128, 2, 512], BF16, tag="sigc")
        for ct in range(2):
            nc.vector.tensor_scalar(sigc[:, ct], sig[:],
                                    spectral[:, ct:ct + 1], None,
                                    op0=ALU.mult)
        rcp = ps_rec.tile([128, 2, 512], F32, tag="rc")
        nc.tensor.matmul(rcp[:, 0], MT[:, 0:128],
                         G[:, pt * 512:(pt + 1) * 512], start=True, stop=True)
        nc.tensor.matmul(rcp[:, 1], MT[:, 128:256],
                         G[:, pt * 512:(pt + 1) * 512], start=True, stop=True)
        R = ew.tile([128, 2, 512], BF16, tag="R")
        for ct in range(2):
            nc.scalar.activation(R[:, ct], rcp[:, ct], AF.Identity,
                                 bias=brn_sb[:, ct:ct + 1])
        E = ew.tile([128, 2, 512], BF16, tag="E")
        nc.vector.tensor_tensor(E[:], Dall[:, pt], sigc[:], op=ALU.mult)
        fu = outr.tile([128, 2, 512], BF16, tag="fu")
        nc.vector.tensor_tensor(fu[:], E[:], Qall[:, pt], op=ALU.add)
        E2 = ew.tile([128, 2, 512], BF16, tag="E2")
        nc.vector.tensor_tensor(E2[:], R[:], sigc[:], op=ALU.mult)
        cp = outr.tile([128, 2, 512], BF16, tag="cp")
        nc.gpsimd.tensor_tensor(cp[:], E2[:], fm_sb[:, pt], op=ALU.add)
        for ct in range(2):
            nc.sync.dma_start(
                fo_v[:, ct, pt * 8:(pt + 1) * 8, :],
                fu[:, ct].rearrange("p (h w) -> p h w", h=8))
            nc.sync.dma_start(
                co_v[:, ct, pt * 8:(pt + 1) * 8, :],
                cp[:, ct].rearrange("p (h w) -> p h w", h=8))
    ctx.close()


def _prep_weights(W3, b3, Wa1, ba1, Wa2, ba2, Wa3, ba3, Wu, bu, Wr, br,
                  Wsa, bsa, Wsm, bsm, lam):
    f = np.float32
    bf = ml_dtypes.bfloat16
    # w3t[p, kt, t, co] = W3[co, kt*128+p, dy, dx]
    w3t = np.ascontiguousarray(
        W3.reshape(C, 4, 128, 9).transpose(2, 1, 3, 0)).astype(bf)
    b3h = b3.reshape(2, 128).T
    # wrt[p, kk, mm, m] = Wr[mm*128+m, kk*128+p]
    wrt = Wr.reshape(2, 128, 2, 128).transpose(3, 2, 0, 1).reshape(128, 512)
    brn = br.reshape(2, 128).T
    # wsc[p, kk, mm, m]: kk<2 -> Wsa/128 (mean folded), kk>=2 -> Wsm
    wsa_r = (Wsa / 128.0).reshape(2, 128, 2, 128).transpose(3, 2, 0, 1)
    wsm_r = Wsm.reshape(2, 128, 2, 128).transpose(3, 2, 0, 1)
    wsc = np.concatenate([wsa_r, wsm_r], axis=1).reshape(128, 1024)
    bsc = (bsa + bsm).reshape(2, 128).T
    smallw = np.ascontiguousarray(np.concatenate(
        [wrt, wsc, b3h, brn, bsc], axis=1)).astype(f)
    # tiny[r, :] = [bum(3) | lam(1) | wum(24, row0 only) | wub1(12, row0)]
    bum = np.stack([Wu @ ba1 + bu, Wu @ ba2 + bu, Wu @ ba3 + bu], axis=1)
    wum = np.stack([(Wu @ Wa1).T, (Wu @ Wa2).T, (Wu @ Wa3).T], axis=0)
    wuwa1 = Wu @ Wa1
    wub1 = np.concatenate([wuwa1[:, 0], wuwa1[:, 1], Wu @ ba1 + bu])
    tiny = np.zeros((4, 40), f)
    tiny[:, 0:3] = bum
    tiny[:, 3] = np.asarray(lam).reshape(4)
    tiny[0, 4:28] = wum.reshape(24)
    tiny[0, 28:40] = wub1
    return dict(w3t=w3t, smallw=smallw, tiny=tiny)


_CACHE = {}


def kernel(frm_feat, other_feat, W3, b3, Wa1, ba1, Wa2, ba2, Wa3, ba3,
           Wu, bu, Wr, br, ws, bs, Wsa, bsa, Wsm, bsm, alpha, lam,
           _trace=False, _tmpdir=None):
    frm_feat = np.asarray(frm_feat, np.float32)
    other_feat = np.asarray(other_feat, np.float32)
    key = (float(alpha), float(ws), float(bs))
    if key not in _CACHE:
        _CACHE[key] = build_program(float(alpha), float(ws), float(bs))
    nc = _CACHE[key]

    wd = _prep_weights(np.asarray(W3), np.asarray(b3), np.asarray(Wa1),
                       np.asarray(ba1), np.asarray(Wa2), np.asarray(ba2),
                       np.asarray(Wa3), np.asarray(ba3), np.asarray(Wu),
                       np.asarray(bu), np.asarray(Wr), np.asarray(br),
                       np.asarray(Wsa), np.asarray(bsa), np.asarray(Wsm),
                       np.asarray(bsm), np.asarray(lam))

    in_maps = []
    for b_i in range(NCORES):
        m = dict(wd)
        m["frm"] = np.ascontiguousarray(frm_feat[b_i])
        m["oth"] = np.ascontiguousarray(other_feat[b_i])
        in_maps.append(m)

    res = bass_utils.run_bass_kernel_spmd(
        nc, in_maps, core_ids=list(range(NCORES)), trace=_trace,
        tmpdir=_tmpdir)
    fused = np.stack([np.asarray(res.results[i]["fused"])
                      for i in range(NCORES)]).astype(np.float32)
    cpr = np.stack([np.asarray(res.results[i]["cpr"])
                    for i in range(NCORES)]).astype(np.float32)
    kernel._last_exec_time_ns = res.exec_time_ns
    kernel._last_results = res
    return fused, cpr


# revision 51
# speedup vs baseline: 1.1778x; 1.0325x over previous
"""Trainium2 Bass kernel for nn_MDRMWithCPRecon.

Sharding: pure data parallel over batch B=8 -> one batch element per
NeuronCore (8 cores). All parameters replicated.

v2 changes vs baseline (294us):
  - bf16 conv (FWL weight loads, 216ns/MM vs fp32r 239ns), bf16
    everywhere downstream (DVE 2x elementwise, halved output DMA).
  - input staged in 16 quarter-chunks, converted f32->bf16 on
    scalar/vector engines, conv starts after first chunk (~7us vs 35us).
  - Fm kept in SBUF (kills the 8MB DRAM round trip).
  - U_gen folded host-side: u = (Wu@Wa) @ [avg;max] + (Wu@ba+bu) -> the
    whole adapter stage disappears.
  - recon bias br folded as a 5th row of G / MT.
  - final stage: ct-pairs processed in single wide ops, spread across
    scalar (Q, R, sig*spectral), vector (D, E, fu, E2) and gpsimd (cp).
"""

import numpy as np
import ml_dtypes

import concourse.bacc as bacc
import concourse.bass as bass
import concourse.tile as tile
from concourse import mybir, bass_utils

F32 = mybir.dt.float32
BF16 = mybir.dt.bfloat16
AF = mybir.ActivationFunctionType
ALU = mybir.AluOpType
AX = mybir.AxisListType

B, C, H, W, K = 8, 256, 64, 64, 4
HW = H * W
NCORES = 8


def build_program(alpha, ws, bs):
    from concourse.masks import make_identity

    nc = bacc.Bacc("TRN2", target_bir_lowering=False, debug=False,
                   num_devices=NCORES)

    frm = nc.dram_tensor("frm", [C, H, W], F32, kind="ExternalInput")
    oth = nc.dram_tensor("oth", [C, H, W], F32, kind="ExternalInput")
    w3t_d = nc.dram_tensor("w3t", [128, 2, 4, 9, 128], BF16,
                           kind="ExternalInput")
    # all small weights packed into two tensors (one DMA each):
    # smallw[p, :] = [wrt(512) | wsc(1024) | b3(2) | brn(2) | bsc(2)]
    # tiny[r, :]   = [bum(3) | lam(1) | wum(24, row0) | wub1(12, row0)]
    smallw_d = nc.dram_tensor("smallw", [128, 1542], F32, kind="ExternalInput")
    tiny_d = nc.dram_tensor("tiny", [4, 40], F32, kind="ExternalInput")
    fused_o = nc.dram_tensor("fused", [C, H, W], BF16, kind="ExternalOutput")
    cpr_o = nc.dram_tensor("cpr", [C, H, W], BF16, kind="ExternalOutput")

    with tile.TileContext(nc) as tc:
        _build_tile(tc, nc, make_identity, locals(), alpha, ws, bs)
    nc.compile()
    return nc


def _build_tile(tc, nc, make_identity, T, alpha, ws, bs):
    frm, oth = T["frm"], T["oth"]
    w3t_d, smallw_d, tiny_d = T["w3t_d"], T["smallw_d"], T["tiny_d"]
    fused_o, cpr_o = T["fused_o"], T["cpr_o"]

    import contextlib
    ctx = contextlib.ExitStack()
    consts = ctx.enter_context(tc.tile_pool(name="consts", bufs=1))
    stage = ctx.enter_context(tc.tile_pool(name="stage", bufs=4))
    ew = ctx.enter_context(tc.tile_pool(name="ew", bufs=2))
    outr = ctx.enter_context(tc.tile_pool(name="outr", bufs=2))
    ps_conv = ctx.enter_context(tc.tile_pool(name="ps_conv", bufs=2, space="PSUM"))
    ps_sm = ctx.enter_context(tc.tile_pool(name="ps_sm", bufs=2, space="PSUM"))
    ps_spat = ctx.enter_context(tc.tile_pool(name="ps_spat", bufs=2, space="PSUM"))
    ps_rec = ctx.enter_context(tc.tile_pool(name="ps_rec", bufs=1, space="PSUM"))

    # ---- conv weights on the scalar queue, ct0 slices first so the first
    # conv tile's weights land ASAP (each (ch,kt) slice is contiguous) ----
    w3_sb = consts.tile([128, 2, 4, 9, 128], BF16)
    for ch in range(2):
        for kt in range(4):
            nc.scalar.dma_start(w3_sb[:, ch, kt], w3t_d[:, ch, kt])

    # ---- packed small weights (issued later; not needed until U-chain) ---
    smallw_sb = consts.tile([128, 1542], F32)
    tiny_sb = consts.tile([4, 40], F32)
    wrt_sb = smallw_sb[:, 0:512].rearrange("p (kk mm m) -> p kk mm m", kk=2,
                                           mm=2)
    wsc_sb = smallw_sb[:, 512:1536].rearrange("p (kk mm m) -> p kk mm m",
                                              kk=4, mm=2)
    b3_sb = smallw_sb[:, 1536:1538]
    brn_sb = smallw_sb[:, 1538:1540]
    bsc_sb = smallw_sb[:, 1540:1542]
    bum_sb = tiny_sb[:, 0:3]
    lam_sb = tiny_sb[:, 3:4]
    wum_sb = tiny_sb[0:1, 4:28].rearrange("p (m s r) -> p m s r", m=3, s=2)
    wub1_sb = tiny_sb[0:1, 28:40]

    MT = consts.tile([4, 256], BF16)

    ident = consts.tile([128, 128], F32)
    make_identity(nc, ident[:])
    ones128 = consts.tile([128, 1], F32)
    nc.gpsimd.memset(ones128[:], 1.0)
    ones4 = consts.tile([4, 128], BF16)
    nc.gpsimd.memset(ones4[:], 1.0)
    ones_row = consts.tile([4, 128], F32)
    nc.gpsimd.memset(ones_row[:], 1.0)
    G = consts.tile([4, HW], BF16)          # CP factor outer products

    # gpsimd tensor_tensor ucode warm-up (IRAM load off the critical path)
    scrap3 = consts.tile([4, 8], BF16)
    nc.gpsimd.tensor_tensor(scrap3[:], ones4[:, 0:8], ones4[:, 8:16],
                            op=ALU.add)

    # ---- padded bf16 image [128, 4(kt), 66, 66]; kt 0/1 frm, 2/3 oth ----
    xr = consts.tile([128, 4, 66, 66], BF16)
    for kt in range(4):
        nc.vector.memset(xr[:, kt, 0:1, :], 0.0)
        nc.vector.memset(xr[:, kt, 65:66, :], 0.0)
        nc.vector.memset(xr[:, kt, 1:65, 0:1], 0.0)
        nc.vector.memset(xr[:, kt, 1:65, 65:66], 0.0)

    # stream input in 16 quarter-chunks over the 3 DMA-capable queues
    # (sync/gpsimd early; scalar queue is busy with w3). Convert f32->bf16
    # on DVE early (idle then) and ACT late (DVE is stats-busy by then).
    n_cv = 0
    for q in range(4):
        for kt in range(4):
            src = frm if kt < 2 else oth
            kt2 = kt % 2
            src_v = src.rearrange("(k p) h w -> p k h w", p=128)
            stg = stage.tile([128, 16, 64], F32, tag="stg")
            if q < 2:
                eng = nc.sync if n_cv % 2 == 0 else nc.gpsimd
            else:
                eng = (nc.sync, nc.gpsimd, nc.scalar)[n_cv % 3]
            eng.dma_start(stg[:], src_v[:, kt2, q * 16:(q + 1) * 16, :])
            dst = xr[:, kt, 1 + q * 16: 1 + (q + 1) * 16, 1:65]
            if q < 2:
                nc.vector.tensor_copy(dst, stg[:])
            else:
                nc.scalar.copy(dst, stg[:])
            n_cv += 1

    # packed small weights (after the input chunks on the sync queue)
    nc.sync.dma_start(smallw_sb[:], smallw_d[:])
    nc.sync.dma_start(tiny_sb[:], tiny_d[:])

    # preload ACT tables now (runs during conv, before the U-chain needs
    # Exp/Sigmoid/Identity)
    scrap = consts.tile([1, 8], F32)
    nc.vector.memset(scrap[:], 0.0)
    scrap2 = consts.tile([1, 8], F32)
    nc.scalar.activation(scrap2[0:1, 2:4], scrap[0:1, 2:4], AF.Exp)
    nc.scalar.activation(scrap2[0:1, 4:6], scrap[0:1, 4:6], AF.Sigmoid)
    nc.scalar.activation(scrap2[0:1, 6:8], scrap[0:1, 6:8], AF.Identity)

    # ---- pooled-stat tiles ----
    sums1 = consts.tile([128, 2, 8], F32)       # per-(ct,pt) sums of Fm
    q_sum = consts.tile([128, 2, 64], F32)      # sum over w  -> [c, h]
    q_max = consts.tile([128, 2, 64], F32)
    # ping-pong accumulators for the over-h stats (combined during conv)
    ppA = consts.tile([128, 2, 2, 64], F32)     # [sum/max, ct, w] bank A
    ppB = consts.tile([128, 2, 2, 64], F32)
    # pt-major so [:, pt] slices are contiguous (keeps DVE 2x bf16 mode)
    fm_sb = consts.tile([128, 8, 2, 512], BF16)  # Fm resident in SBUF
    Qall = consts.tile([128, 8, 2, 512], BF16)   # (1-a)*oth per tile
    Dall = consts.tile([128, 8, 2, 512], BF16)   # a*frm - Q per tile

    a = float(alpha)

    # ---- conv3x3 + leaky relu + streaming stats + Q/D precompute ----
    for pt in range(8):
        for ct in range(2):
            ps = ps_conv.tile([128, 512], F32, tag="conv")
            idx = 0
            for kt in range(4):
                for t in range(9):
                    dy, dx = t // 3, t % 3
                    nc.tensor.matmul(
                        ps[:],
                        w3_sb[:, ct, kt, t],
                        xr[:, kt, pt * 8 + dy: pt * 8 + dy + 8, dx: dx + 64],
                        start=(idx == 0), stop=(idx == 35))
                    idx += 1
            nc.scalar.activation(fm_sb[:, pt, ct], ps[:], AF.Lrelu,
                                 bias=b3_sb[:, ct:ct + 1], alpha=0.01,
                                 accum_out=sums1[:, ct, pt:pt + 1])
            blk = fm_sb[:, pt, ct].rearrange("p (h w) -> p h w", h=8)
            blk_t = fm_sb[:, pt, ct].rearrange("p (h w) -> p w h", h=8)
            nc.vector.tensor_reduce(q_sum[:, ct, pt * 8:(pt + 1) * 8], blk,
                                    axis=AX.X, op=ALU.add)
            nc.vector.tensor_reduce(q_max[:, ct, pt * 8:(pt + 1) * 8], blk,
                                    axis=AX.X, op=ALU.max)
            src_pp, dst_pp = (ppA, ppB) if pt % 2 else (ppB, ppA)
            if pt == 0:
                nc.vector.tensor_reduce(ppA[:, 0, ct, :], blk_t,
                                        axis=AX.X, op=ALU.add)
                nc.vector.tensor_reduce(ppA[:, 1, ct, :], blk_t,
                                        axis=AX.X, op=ALU.max)
            else:
                ppt = ew.tile([128, 2, 64], F32, tag="ppt")
                nc.vector.tensor_reduce(ppt[:, 0, :], blk_t,
                                        axis=AX.X, op=ALU.add)
                nc.vector.tensor_reduce(ppt[:, 1, :], blk_t,
                                        axis=AX.X, op=ALU.max)
                nc.vector.tensor_tensor(dst_pp[:, 0, ct, :],
                                        src_pp[:, 0, ct, :], ppt[:, 0, :],
                                        op=ALU.add)
                nc.vector.tensor_tensor(dst_pp[:, 1, ct, :],
                                        src_pp[:, 1, ct, :], ppt[:, 1, :],
                                        op=ALU.max)
            # Q/D for the final stage (only need xr; hide under conv)
            oth_t = xr[:, 2 + ct, 1 + pt * 8: 9 + pt * 8, 1:65]
            frm_t = xr[:, ct, 1 + pt * 8: 9 + pt * 8, 1:65]
            nc.scalar.activation(
                Qall[:, pt, ct].rearrange("p (h w) -> p h w", h=8), oth_t,
                AF.Copy, scale=float(1.0 - a))
            nc.vector.scalar_tensor_tensor(
                Dall[:, pt, ct].rearrange("p (h w) -> p h w", h=8), frm_t, a,
                Qall[:, pt, ct].rearrange("p (h w) -> p h w", h=8),
                op0=ALU.mult, op1=ALU.subtract)

    # ---- combine remaining partials (pp ended in ppB after 7 adds) ----
    p_sum = ppB[:, 0]       # [128, 2, 64]
    p_max = ppB[:, 1]
    sm1 = consts.tile([128, 4], F32)    # [sum1 ct0, ct1, max1 ct0, ct1]
    nc.vector.tensor_reduce(sm1[:, 0:2], sums1[:], axis=AX.X, op=ALU.add)
    nc.vector.tensor_reduce(sm1[:, 2:4], q_max[:], axis=AX.X, op=ALU.max)

    # ---- pooled rows (modes 2/3): rows_sb[0, s, :] = [m2(64) | m3(64)] ----
    rows_sb = consts.tile([1, 2, 128], F32)

    def colrows(S_sum, S_max, denom, off):
        ssum = ps_sm.tile([1, 64], F32, tag="sm")
        nc.tensor.matmul(ssum[:], ones128[:], S_sum[:, 0, :], start=True,
                         stop=False)
        nc.tensor.matmul(ssum[:], ones128[:], S_sum[:, 1, :], start=False,
                         stop=True)
        nc.scalar.mul(rows_sb[0:1, 0, off:off + 64], ssum[:], 1.0 / denom)
        tpm = ps_sm.tile([128, 128], F32, tag="sm")
        nc.tensor.transpose(tpm[:], S_max[:].rearrange("p c w -> p (c w)"),
                            ident[:])
        mxr = ew.tile([128, 1], F32, tag="mxr")
        nc.vector.tensor_reduce(mxr[:], tpm[:], axis=AX.X, op=ALU.max)
        tpb = ps_sm.tile([1, 128], F32, tag="sm")
        nc.tensor.transpose(tpb[:], mxr[:], ident[:])
        nc.vector.tensor_reduce(
            rows_sb[0:1, 1, off:off + 64],
            tpb[:].rearrange("p (c w) -> p w c", c=2), axis=AX.X, op=ALU.max)

    colrows(p_sum, p_max, C * H, 0)
    colrows(q_sum, q_max, C * W, 64)

    U1T = consts.tile([4, 256], BF16)
    uct = consts.tile([4, 128], BF16)    # [U2T(64) | U3T(64)]
    U1n = consts.tile([128, 2, 4], F32)

    def softmax4(src, cw, dst_n):
        """src [cw, 4] logits -> dst_n [cw, 4] softmax; returns dst_n."""
        negm = ew.tile([cw, 1], F32, tag="negm")
        nc.vector.tensor_reduce(negm[:], src, axis=AX.X, op=ALU.max,
                                negate=True)
        ssum = ew.tile([cw, 1], F32, tag="ssum")
        etile = ew.tile([cw, 4], F32, tag="etile")
        nc.scalar.activation(etile[:], src, AF.Exp, bias=negm[:],
                             accum_out=ssum[:])
        rec = ew.tile([cw, 1], F32, tag="rec")
        nc.vector.reciprocal(rec[:], ssum[:])
        nc.vector.tensor_scalar(dst_n, etile[:], rec[:], None, op0=ALU.mult)
        return dst_n

    # mode 1: logits computed directly channel-on-partition:
    # u1t[c, r] = wb0[r]*avg1[c] + wb1[r]*max1[c] + bum1[r]
    bc_ps = ps_sm.tile([128, 12], F32, tag="sm")
    nc.tensor.matmul(bc_ps[:], ones_row[0:1, :], wub1_sb[:], start=True,
                     stop=True)
    bc_sb = consts.tile([128, 12], F32)
    nc.scalar.mul(bc_sb[:], bc_ps[:], 1.0)
    for ct in range(2):
        s1 = ew.tile([128, 4], F32, tag="s1")
        nc.vector.scalar_tensor_tensor(s1[:], bc_sb[:, 0:4],
                                       sm1[:, ct:ct + 1], bc_sb[:, 8:12],
                                       op0=ALU.mult, op1=ALU.add)
        u1t = ew.tile([128, 4], F32, tag="u1t")
        nc.vector.scalar_tensor_tensor(u1t[:], bc_sb[:, 4:8],
                                       sm1[:, 2 + ct:3 + ct], s1[:],
                                       op0=ALU.mult, op1=ALU.add)
        softmax4(u1t[:], 128, U1n[:, ct, :])
        tb_ps = ps_sm.tile([4, 128], F32, tag="sm")
        nc.tensor.transpose(tb_ps[:], U1n[:, ct, :], ident[:])
        nc.scalar.copy(U1T[:, ct * 128:(ct + 1) * 128], tb_ps[:])

    # modes 2/3: rank-1 matmuls from pooled rows, transpose, softmax
    for m, off in ((1, 0), (2, 64)):
        u_ps = ps_sm.tile([4, 64], F32, tag="sm")
        nc.tensor.matmul(u_ps[:], wum_sb[0:1, m, 0, :],
                         rows_sb[0:1, 0, off:off + 64], start=True, stop=False)
        nc.tensor.matmul(u_ps[:], wum_sb[0:1, m, 1, :],
                         rows_sb[0:1, 1, off:off + 64], start=False, stop=True)
        u_sb = consts.tile([4, 64], F32, tag=f"u{m}")
        nc.scalar.activation(u_sb[:], u_ps[:], AF.Identity,
                             bias=bum_sb[:, m:m + 1])
        ut_ps = ps_sm.tile([64, 4], F32, tag="sm")
        nc.tensor.transpose(ut_ps[:], u_sb[:], ident[0:4, 0:4])
        sm_t = ew.tile([64, 4], F32, tag="smt")
        softmax4(ut_ps[:], 64, sm_t[:])
        tb_ps = ps_sm.tile([4, 64], F32, tag="sm")
        nc.tensor.transpose(tb_ps[:], sm_t[:], ident[0:64, 0:64])
        nc.scalar.copy(uct[:, (m - 1) * 64:m * 64], tb_ps[:])

    # ---- G: G[r, h, w] = U3T[r,h]*U2T[r,w] (DVE/GPS halves) ----
    Gv = G[:].rearrange("p (h w) -> p h w", h=64)
    nc.vector.tensor_tensor(
        Gv[:, 0:32, :],
        uct[0:4, 64:96][:, :, None].broadcast_to([4, 32, 64]),
        uct[0:4, 0:64][:, None, :].broadcast_to([4, 32, 64]), op=ALU.mult)
    nc.gpsimd.tensor_tensor(
        Gv[:, 32:64, :],
        uct[0:4, 96:128][:, :, None].broadcast_to([4, 32, 64]),
        uct[0:4, 0:64][:, None, :].broadcast_to([4, 32, 64]), op=ALU.mult)

    # ---- MT rows 0-3 = (Wr @ U1 diag(lam))^T ----
    for mm in range(2):
        m_ps = ps_sm.tile([128, 4], F32, tag="sm")
        for kk in range(2):
            nc.tensor.matmul(m_ps[:], wrt_sb[:, kk, mm, :], U1n[:, kk, :],
                             start=(kk == 0), stop=(kk == 1))
        m_sb = ew.tile([128, 4], F32, tag="msb")
        nc.scalar.copy(m_sb[:], m_ps[:])
        mt_ps = ps_sm.tile([4, 128], F32, tag="sm")
        nc.tensor.transpose(mt_ps[:], m_sb[:], ident[:])
        nc.vector.tensor_scalar(MT[:, mm * 128:(mm + 1) * 128], mt_ps[:],
                                lam_sb[:], None, op0=ALU.mult)

    # ---- spectral attention ----
    gag = consts.tile([128, 4], F32)  # [ga_ct0, ga_ct1, gm_ct0, gm_ct1]
    for ct in range(2):
        f_ps = ps_sm.tile([128, 128], F32, tag="sm")
        nc.tensor.matmul(f_ps[:], U1T[:, ct * 128:(ct + 1) * 128], uct[:],
                         start=True, stop=True)
        nc.vector.tensor_reduce(gag[:, ct:ct + 1], f_ps[:], axis=AX.X,
                                op=ALU.add)
        nc.vector.tensor_reduce(gag[:, 2 + ct:3 + ct], f_ps[:], axis=AX.X,
                                op=ALU.max)
    spectral = consts.tile([128, 2], F32)
    for mm in range(2):
        sp_ps = ps_sm.tile([128, 1], F32, tag="sm")
        for kk in range(4):
            nc.tensor.matmul(sp_ps[:], wsc_sb[:, kk, mm, :],
                             gag[:, kk:kk + 1], start=(kk == 0),
                             stop=(kk == 3))
        stmp = ew.tile([128, 1], F32, tag="stmp")
        nc.scalar.activation(stmp[:], sp_ps[:], AF.Sigmoid,
                             bias=bsc_sb[:, mm:mm + 1])
        nc.scalar.activation(spectral[:, mm:mm + 1], stmp[:], AF.Sigmoid)

    # ---- final elementwise stage ----
    # fused = D*sigc + Q;  cp_recon = (rc + br)*sigc + Fm; sigc = spc*sig
    fo_v = fused_o.rearrange("(c p) h w -> p c h w", p=128)
    co_v = cpr_o.rearrange("(c p) h w -> p c h w", p=128)
    for pt in range(8):
        sp_ps = ps_spat.tile([128, 512], F32, tag="spat")
        nc.tensor.matmul(sp_ps[:], ones4[:], G[:, pt * 512:(pt + 1) * 512],
                         start=True, stop=True)
        sig = ew.tile([128, 512], BF16, tag="sig")
        nc.scalar.activation(sig[:], sp_ps[:], AF.Sigmoid,
                             scale=float(ws), bias=float(bs))
        sigc = ew.tile([128, 2, 512], BF16, tag="sigc")
        for ct in range(2):
            nc.vector.tensor_scalar(sigc[:, ct], sig[:],
                                    spectral[:, ct:ct + 1], None,
                                    op0=ALU.mult)
        rcp = ps_rec.tile([128, 2, 512], F32, tag="rc")
        nc.tensor.matmul(rcp[:, 0], MT[:, 0:128],
                         G[:, pt * 512:(pt + 1) * 512], start=True, stop=True)
        nc.tensor.matmul(rcp[:, 1], MT[:, 128:256],
                         G[:, pt * 512:(pt + 1) * 512], start=True, stop=True)
        R = ew.tile([128, 2, 512], BF16, tag="R")
        for ct in range(2):
            nc.scalar.activation(R[:, ct], rcp[:, ct], AF.Identity,
                                 bias=brn_sb[:, ct:ct + 1])
        E = ew.tile([128, 2, 512], BF16, tag="E")
        nc.vector.tensor_tensor(E[:], Dall[:, pt], sigc[:], op=ALU.mult)
        fu = outr.tile([128, 2, 512], BF16, tag="fu")
        nc.vector.tensor_tensor(fu[:], E[:], Qall[:, pt], op=ALU.add)
        E2 = ew.tile([128, 2, 512], BF16, tag="E2")
        nc.vector.tensor_tensor(E2[:], R[:], sigc[:], op=ALU.mult)
        cp = outr.tile([128, 2, 512], BF16, tag="cp")
        nc.gpsimd.tensor_tensor(cp[:], E2[:], fm_sb[:, pt], op=ALU.add)
        for ct in range(2):
            nc.sync.dma_start(
                fo_v[:, ct, pt * 8:(pt + 1) * 8, :],
                fu[:, ct].rearrange("p (h w) -> p h w", h=8))
            nc.sync.dma_start(
                co_v[:, ct, pt * 8:(pt + 1) * 8, :],
                cp[:, ct].rearrange("p (h w) -> p h w", h=8))
    ctx.close()


def _prep_weights(W3, b3, Wa1, ba1, Wa2, ba2, Wa3, ba3, Wu, bu, Wr, br,
                  Wsa, bsa, Wsm, bsm, lam):
    f = np.float32
    bf = ml_dtypes.bfloat16
    # w3t[p, ch, kt, t, co'] = W3[ch*128+co', kt*128+p, dy, dx]
    w3t = np.ascontiguousarray(
        W3.reshape(2, 128, 4, 128, 9).transpose(3, 0, 2, 4, 1)).astype(bf)
    b3h = b3.reshape(2, 128).T
    # wrt[p, kk, mm, m] = Wr[mm*128+m, kk*128+p]
    wrt = Wr.reshape(2, 128, 2, 128).transpose(3, 2, 0, 1).reshape(128, 512)
    brn = br.reshape(2, 128).T
    # wsc[p, kk, mm, m]: kk<2 -> Wsa/128 (mean folded), kk>=2 -> Wsm
    wsa_r = (Wsa / 128.0).reshape(2, 128, 2, 128).transpose(3, 2, 0, 1)
    wsm_r = Wsm.reshape(2, 128, 2, 128).transpose(3, 2, 0, 1)
    wsc = np.concatenate([wsa_r, wsm_r], axis=1).reshape(128, 1024)
    bsc = (bsa + bsm).reshape(2, 128).T
    smallw = np.ascontiguousarray(np.concatenate(
        [wrt, wsc, b3h, brn, bsc], axis=1)).astype(f)
    # tiny[r, :] = [bum(3) | lam(1) | wum(24, row0 only) | wub1(12, row0)]
    bum = np.stack([Wu @ ba1 + bu, Wu @ ba2 + bu, Wu @ ba3 + bu], axis=1)
    wum = np.stack([(Wu @ Wa1).T, (Wu @ Wa2).T, (Wu @ Wa3).T], axis=0)
    wuwa1 = Wu @ Wa1
    wub1 = np.concatenate([wuwa1[:, 0], wuwa1[:, 1], Wu @ ba1 + bu])
    tiny = np.zeros((4, 40), f)
    tiny[:, 0:3] = bum
    tiny[:, 3] = np.asarray(lam).reshape(4)
    tiny[0, 4:28] = wum.reshape(24)
    tiny[0, 28:40] = wub1
    return dict(w3t=w3t, smallw=smallw, tiny=tiny)


_CACHE = {}


def kernel(frm_feat, other_feat, W3, b3, Wa1, ba1, Wa2, ba2, Wa3, ba3,
           Wu, bu, Wr, br, ws, bs, Wsa, bsa, Wsm, bsm, alpha, lam,
           _trace=False, _tmpdir=None):
    frm_feat = np.asarray(frm_feat, np.float32)
    other_feat = np.asarray(other_feat, np.float32)
    key = (float(alpha), float(ws), float(bs))
    if key not in _CACHE:
        _CACHE[key] = build_program(float(alpha), float(ws), float(bs))
    nc = _CACHE[key]

    wd = _prep_weights(np.asarray(W3), np.asarray(b3), np.asarray(Wa1),
                       np.asarray(ba1), np.asarray(Wa2), np.asarray(ba2),
                       np.asarray(Wa3), np.asarray(ba3), np.asarray(Wu),
                       np.asarray(bu), np.asarray(Wr), np.asarray(br),
                       np.asarray(Wsa), np.asarray(bsa), np.asarray(Wsm),
                       np.asarray(bsm), np.asarray(lam))

    in_maps = []
    for b_i in range(NCORES):
        m = dict(wd)
        m["frm"] = np.ascontiguousarray(frm_feat[b_i])
        m["oth"] = np.ascontiguousarray(other_feat[b_i])
        in_maps.append(m)

    res = bass_utils.run_bass_kernel_spmd(
        nc, in_maps, core_ids=list(range(NCORES)), trace=_trace,
        tmpdir=_tmpdir)
    fused = np.stack([np.asarray(res.results[i]["fused"])
                      for i in range(NCORES)]).astype(np.float32)
    cpr = np.stack([np.asarray(res.results[i]["cpr"])
                    for i in range(NCORES)]).astype(np.float32)
    kernel._last_exec_time_ns = res.exec_time_ns
    kernel._last_results = res
    return fused, cpr


# revision 62
# speedup vs baseline: 1.1872x; 1.0080x over previous
"""Trainium2 Bass kernel for nn_MDRMWithCPRecon.

Sharding: pure data parallel over batch B=8 -> one batch element per
NeuronCore (8 cores). All parameters replicated.

v2 changes vs baseline (294us):
  - bf16 conv (FWL weight loads, 216ns/MM vs fp32r 239ns), bf16
    everywhere downstream (DVE 2x elementwise, halved output DMA).
  - input staged in 16 quarter-chunks, converted f32->bf16 on
    scalar/vector engines, conv starts after first chunk (~7us vs 35us).
  - Fm kept in SBUF (kills the 8MB DRAM round trip).
  - U_gen folded host-side: u = (Wu@Wa) @ [avg;max] + (Wu@ba+bu) -> the
    whole adapter stage disappears.
  - recon bias br folded as a 5th row of G / MT.
  - final stage: ct-pairs processed in single wide ops, spread across
    scalar (Q, R, sig*spectral), vector (D, E, fu, E2) and gpsimd (cp).
"""

import numpy as np
import ml_dtypes

import concourse.bacc as bacc
import concourse.bass as bass
import concourse.tile as tile
from concourse import mybir, bass_utils

F32 = mybir.dt.float32
BF16 = mybir.dt.bfloat16
AF = mybir.ActivationFunctionType
ALU = mybir.AluOpType
AX = mybir.AxisListType

B, C, H, W, K = 8, 256, 64, 64, 4
HW = H * W
NCORES = 8


def build_program(alpha, ws, bs):
    from concourse.masks import make_identity

    nc = bacc.Bacc("TRN2", target_bir_lowering=False, debug=False,
                   num_devices=NCORES)

    frm = nc.dram_tensor("frm", [C, H, W], F32, kind="ExternalInput")
    oth = nc.dram_tensor("oth", [C, H, W], F32, kind="ExternalInput")
    w3t_d = nc.dram_tensor("w3t", [128, 2, 4, 9, 128], BF16,
                           kind="ExternalInput")
    # all small weights packed into two tensors (one DMA each):
    # smallw[p, :] = [wrt(512) | wsc(1024) | b3(2) | brn(2) | bsc(2)]
    # tiny[r, :]   = [bum(3) | lam(1) | wum(24, row0) | wub1(12, row0)]
    smallw_d = nc.dram_tensor("smallw", [128, 1542], F32, kind="ExternalInput")
    tiny_d = nc.dram_tensor("tiny", [4, 52], F32, kind="ExternalInput")
    brrow_d = nc.dram_tensor("brrow", [1, 256], BF16, kind="ExternalInput")
    fused_o = nc.dram_tensor("fused", [C, H, W], BF16, kind="ExternalOutput")
    cpr_o = nc.dram_tensor("cpr", [C, H, W], BF16, kind="ExternalOutput")

    with tile.TileContext(nc) as tc:
        _build_tile(tc, nc, make_identity, locals(), alpha, ws, bs)
    nc.compile()
    return nc


def _build_tile(tc, nc, make_identity, T, alpha, ws, bs):
    frm, oth = T["frm"], T["oth"]
    w3t_d, smallw_d, tiny_d = T["w3t_d"], T["smallw_d"], T["tiny_d"]
    brrow_d = T["brrow_d"]
    fused_o, cpr_o = T["fused_o"], T["cpr_o"]

    import contextlib
    ctx = contextlib.ExitStack()
    consts = ctx.enter_context(tc.tile_pool(name="consts", bufs=1))
    stage = ctx.enter_context(tc.tile_pool(name="stage", bufs=4))
    ew = ctx.enter_context(tc.tile_pool(name="ew", bufs=2))
    outr = ctx.enter_context(tc.tile_pool(name="outr", bufs=2))
    ps_conv = ctx.enter_context(tc.tile_pool(name="ps_conv", bufs=2, space="PSUM"))
    ps_sm = ctx.enter_context(tc.tile_pool(name="ps_sm", bufs=2, space="PSUM"))
    ps_spat = ctx.enter_context(tc.tile_pool(name="ps_spat", bufs=2, space="PSUM"))
    ps_rec = ctx.enter_context(tc.tile_pool(name="ps_rec", bufs=1, space="PSUM"))

    # ---- conv weights on the scalar queue, ct0 slices first so the first
    # conv tile's weights land ASAP (each (ch,kt) slice is contiguous) ----
    w3_sb = consts.tile([128, 2, 4, 9, 128], BF16)
    for ch in range(2):
        for kt in range(4):
            nc.scalar.dma_start(w3_sb[:, ch, kt], w3t_d[:, ch, kt])

    # ---- packed small weights (issued later; not needed until U-chain) ---
    smallw_sb = consts.tile([128, 1542], F32)
    tiny_sb = consts.tile([4, 52], F32)
    brrow_sb = consts.tile([1, 256], BF16)
    wrt_sb = smallw_sb[:, 0:512].rearrange("p (kk mm m) -> p kk mm m", kk=2,
                                           mm=2)
    wsc_sb = smallw_sb[:, 512:1536].rearrange("p (kk mm m) -> p kk mm m",
                                              kk=4, mm=2)
    b3_sb = smallw_sb[:, 1536:1538]
    brn_sb = smallw_sb[:, 1538:1540]
    bsc_sb = smallw_sb[:, 1540:1542]
    lam_sb = tiny_sb[:, 3:4]
    wum_sb = tiny_sb[0:1, 4:28].rearrange("p (m s r) -> p m s r", m=3, s=2)
    wub1_sb = tiny_sb[0:1, 28:40]
    bumrow_sb = tiny_sb[0:1, 40:52]    # [1, 3*4] bum rows for bias matmuls

    MT = consts.tile([4, 256], BF16)

    ident = consts.tile([128, 128], F32)
    make_identity(nc, ident[:])
    ones128 = consts.tile([128, 1], F32)
    nc.gpsimd.memset(ones128[:], 1.0)
    ones4 = consts.tile([4, 128], BF16)
    nc.gpsimd.memset(ones4[:], 1.0)
    ones_row = consts.tile([4, 128], F32)
    nc.gpsimd.memset(ones_row[:], 1.0)
    ones512b = consts.tile([4, 512], BF16)
    nc.gpsimd.memset(ones512b[:], 1.0)
    G = consts.tile([4, HW], BF16)          # CP factor outer products

    # gpsimd tensor_tensor ucode warm-up (IRAM load off the critical path)
    scrap3 = consts.tile([4, 8], BF16)
    nc.gpsimd.tensor_tensor(scrap3[:], ones4[:, 0:8], ones4[:, 8:16],
                            op=ALU.add)

    # ---- padded bf16 image [128, 4(kt), 66, 66]; kt 0/1 frm, 2/3 oth ----
    xr = consts.tile([128, 4, 66, 66], BF16)
    for kt in range(4):
        nc.vector.memset(xr[:, kt, 0:1, :], 0.0)
        nc.vector.memset(xr[:, kt, 65:66, :], 0.0)
        nc.vector.memset(xr[:, kt, 1:65, 0:1], 0.0)
        nc.vector.memset(xr[:, kt, 1:65, 65:66], 0.0)

    # stream input in 16 quarter-chunks over the 3 DMA-capable queues
    # (sync/gpsimd early; scalar queue is busy with w3). Convert f32->bf16
    # on DVE early (idle then) and ACT late (DVE is stats-busy by then).
    n_cv = 0
    for q in range(4):
        for kt in range(4):
            src = frm if kt < 2 else oth
            kt2 = kt % 2
            src_v = src.rearrange("(k p) h w -> p k h w", p=128)
            stg = stage.tile([128, 16, 64], F32, tag="stg")
            if q < 2:
                eng = nc.sync if n_cv % 2 == 0 else nc.gpsimd
            else:
                eng = (nc.sync, nc.gpsimd, nc.scalar)[n_cv % 3]
            eng.dma_start(stg[:], src_v[:, kt2, q * 16:(q + 1) * 16, :])
            dst = xr[:, kt, 1 + q * 16: 1 + (q + 1) * 16, 1:65]
            nc.vector.tensor_copy(dst, stg[:])
            n_cv += 1

    # packed small weights (after the input chunks on the sync queue)
    nc.sync.dma_start(smallw_sb[:], smallw_d[:])
    nc.sync.dma_start(tiny_sb[:], tiny_d[:])
    nc.sync.dma_start(brrow_sb[:], brrow_d[:])
    scrap = consts.tile([1, 8], F32)
    nc.vector.memset(scrap[:], 0.0)

    # ---- pooled-stat tiles ----
    sums1 = consts.tile([128, 2, 8], F32)       # per-(ct,pt) sums of Fm
    q_sum = consts.tile([128, 2, 64], F32)      # sum over w  -> [c, h]
    q_max = consts.tile([128, 2, 64], F32)
    # ping-pong accumulators for the over-h stats (combined during conv)
    ppA = consts.tile([128, 2, 2, 64], F32)     # [sum/max, ct, w] bank A
    ppB = consts.tile([128, 2, 2, 64], F32)
    # pt-major so [:, pt] slices are contiguous (keeps DVE 2x bf16 mode)
    fm_sb = consts.tile([128, 8, 2, 512], BF16)  # Fm resident in SBUF
    Qall = consts.tile([128, 8, 2, 512], BF16)   # (1-a)*oth per tile
    Dall = consts.tile([128, 8, 2, 512], BF16)   # a*frm - Q per tile

    a = float(alpha)

    # ---- conv3x3 + leaky relu + streaming stats + Q/D precompute ----
    for pt in range(8):
        for ct in range(2):
            ps = ps_conv.tile([128, 512], F32, tag="conv")
            idx = 0
            for kt in range(4):
                for t in range(9):
                    dy, dx = t // 3, t % 3
                    nc.tensor.matmul(
                        ps[:],
                        w3_sb[:, ct, kt, t],
                        xr[:, kt, pt * 8 + dy: pt * 8 + dy + 8, dx: dx + 64],
                        start=(idx == 0), stop=(idx == 35))
                    idx += 1
            nc.scalar.activation(fm_sb[:, pt, ct], ps[:], AF.Lrelu,
                                 bias=b3_sb[:, ct:ct + 1], alpha=0.01,
                                 accum_out=sums1[:, ct, pt:pt + 1])
            blk = fm_sb[:, pt, ct].rearrange("p (h w) -> p h w", h=8)
            blk_t = fm_sb[:, pt, ct].rearrange("p (h w) -> p w h", h=8)
            nc.vector.tensor_reduce(q_sum[:, ct, pt * 8:(pt + 1) * 8], blk,
                                    axis=AX.X, op=ALU.add)
            nc.vector.tensor_reduce(q_max[:, ct, pt * 8:(pt + 1) * 8], blk,
                                    axis=AX.X, op=ALU.max)
            src_pp, dst_pp = (ppA, ppB) if pt % 2 else (ppB, ppA)
            if pt == 0:
                nc.vector.tensor_reduce(ppA[:, 0, ct, :], blk_t,
                                        axis=AX.X, op=ALU.add)
                nc.vector.tensor_reduce(ppA[:, 1, ct, :], blk_t,
                                        axis=AX.X, op=ALU.max)
            else:
                ppt = ew.tile([128, 2, 64], F32, tag="ppt")
                nc.vector.tensor_reduce(ppt[:, 0, :], blk_t,
                                        axis=AX.X, op=ALU.add)
                nc.vector.tensor_reduce(ppt[:, 1, :], blk_t,
                                        axis=AX.X, op=ALU.max)
                nc.vector.tensor_tensor(dst_pp[:, 0, ct, :],
                                        src_pp[:, 0, ct, :], ppt[:, 0, :],
                                        op=ALU.add)
                nc.vector.tensor_tensor(dst_pp[:, 1, ct, :],
                                        src_pp[:, 1, ct, :], ppt[:, 1, :],
                                        op=ALU.max)
            # Q/D for the final stage (only need xr; hide under conv)
            oth_t = xr[:, 2 + ct, 1 + pt * 8: 9 + pt * 8, 1:65]
            frm_t = xr[:, ct, 1 + pt * 8: 9 + pt * 8, 1:65]
            nc.scalar.activation(
                Qall[:, pt, ct].rearrange("p (h w) -> p h w", h=8), oth_t,
                AF.Copy, scale=float(1.0 - a))
            nc.vector.scalar_tensor_tensor(
                Dall[:, pt, ct].rearrange("p (h w) -> p h w", h=8), frm_t, a,
                Qall[:, pt, ct].rearrange("p (h w) -> p h w", h=8),
                op0=ALU.mult, op1=ALU.subtract)

    # load Exp+Sigmoid ACT tables after the last Lrelu eviction (ScalarE is
    # FIFO, so these run post-conv and are resident for the U-chain; the
    # 2-entry table cache then never thrashes: tail uses Sigmoid+Copy only)
    scrap2 = consts.tile([1, 8], F32)
    nc.scalar.activation(scrap2[0:1, 2:4], scrap[0:1, 2:4], AF.Exp)
    nc.scalar.activation(scrap2[0:1, 4:6], scrap[0:1, 4:6], AF.Sigmoid)

    # ---- combine remaining partials (pp ended in ppB after 7 adds) ----
    p_sum = ppB[:, 0]       # [128, 2, 64]
    p_max = ppB[:, 1]
    sm1 = consts.tile([128, 4], F32)    # [sum1 ct0, ct1, max1 ct0, ct1]
    nc.vector.tensor_reduce(sm1[:, 0:2], sums1[:], axis=AX.X, op=ALU.add)
    nc.vector.tensor_reduce(sm1[:, 2:4], q_max[:], axis=AX.X, op=ALU.max)

    # ---- pooled rows (modes 2/3): rows_sb[0, s, :] = [m2(64) | m3(64)] ----
    rows_sb = consts.tile([1, 2, 128], F32)

    def colrows(S_sum, S_max, denom, off):
        ssum = ps_sm.tile([1, 64], F32, tag="sm")
        nc.tensor.matmul(ssum[:], ones128[:], S_sum[:, 0, :], start=True,
                         stop=False)
        nc.tensor.matmul(ssum[:], ones128[:], S_sum[:, 1, :], start=False,
                         stop=True)
        nc.scalar.mul(rows_sb[0:1, 0, off:off + 64], ssum[:], 1.0 / denom)
        tpm = ps_sm.tile([128, 128], F32, tag="sm")
        nc.tensor.transpose(tpm[:], S_max[:].rearrange("p c w -> p (c w)"),
                            ident[:])
        mxr = ew.tile([128, 1], F32, tag="mxr")
        nc.vector.tensor_reduce(mxr[:], tpm[:], axis=AX.X, op=ALU.max)
        tpb = ps_sm.tile([1, 128], F32, tag="sm")
        nc.tensor.transpose(tpb[:], mxr[:], ident[:])
        nc.vector.tensor_reduce(
            rows_sb[0:1, 1, off:off + 64],
            tpb[:].rearrange("p (c w) -> p w c", c=2), axis=AX.X, op=ALU.max)

    colrows(p_sum, p_max, C * H, 0)
    colrows(q_sum, q_max, C * W, 64)

    U1T = consts.tile([4, 256], BF16)
    uct = consts.tile([4, 128], BF16)    # [U2T(64) | U3T(64)]
    U1n = consts.tile([128, 2, 4], F32)

    def softmax4(src, cw, dst_n):
        """src [cw, 4] logits -> dst_n [cw, 4] softmax; returns dst_n."""
        negm = ew.tile([cw, 1], F32, tag="negm")
        nc.vector.tensor_reduce(negm[:], src, axis=AX.X, op=ALU.max,
                                negate=True)
        ssum = ew.tile([cw, 1], F32, tag="ssum")
        etile = ew.tile([cw, 4], F32, tag="etile")
        nc.scalar.activation(etile[:], src, AF.Exp, bias=negm[:],
                             accum_out=ssum[:])
        rec = ew.tile([cw, 1], F32, tag="rec")
        nc.vector.reciprocal(rec[:], ssum[:])
        nc.vector.tensor_scalar(dst_n, etile[:], rec[:], None, op0=ALU.mult)
        return dst_n

    # mode 1: logits computed directly channel-on-partition:
    # u1t[c, r] = wb0[r]*avg1[c] + wb1[r]*max1[c] + bum1[r]
    bc_ps = ps_sm.tile([128, 12], F32, tag="sm")
    nc.tensor.matmul(bc_ps[:], ones_row[0:1, :], wub1_sb[:], start=True,
                     stop=True)
    bc_sb = consts.tile([128, 12], F32)
    nc.scalar.mul(bc_sb[:], bc_ps[:], 1.0)
    for ct in range(2):
        s1 = ew.tile([128, 4], F32, tag="s1")
        nc.vector.scalar_tensor_tensor(s1[:], bc_sb[:, 0:4],
                                       sm1[:, ct:ct + 1], bc_sb[:, 8:12],
                                       op0=ALU.mult, op1=ALU.add)
        u1t = ew.tile([128, 4], F32, tag="u1t")
        nc.vector.scalar_tensor_tensor(u1t[:], bc_sb[:, 4:8],
                                       sm1[:, 2 + ct:3 + ct], s1[:],
                                       op0=ALU.mult, op1=ALU.add)
        softmax4(u1t[:], 128, U1n[:, ct, :])
        tb_ps = ps_sm.tile([4, 128], F32, tag="sm")
        nc.tensor.transpose(tb_ps[:], U1n[:, ct, :], ident[:])
        nc.scalar.copy(U1T[:, ct * 128:(ct + 1) * 128], tb_ps[:])

    # modes 2/3: rank-1 matmuls from pooled rows (+bias matmul), transpose,
    # softmax
    for m, off in ((1, 0), (2, 64)):
        u_ps = ps_sm.tile([4, 64], F32, tag="sm")
        nc.tensor.matmul(u_ps[:], wum_sb[0:1, m, 0, :],
                         rows_sb[0:1, 0, off:off + 64], start=True, stop=False)
        nc.tensor.matmul(u_ps[:], wum_sb[0:1, m, 1, :],
                         rows_sb[0:1, 1, off:off + 64], start=False, stop=False)
        nc.tensor.matmul(u_ps[:], bumrow_sb[0:1, m * 4:(m + 1) * 4],
                         ones_row[0:1, 0:64], start=False, stop=True)
        u_sb = consts.tile([4, 64], F32, tag=f"u{m}")
        nc.scalar.copy(u_sb[:], u_ps[:])
        ut_ps = ps_sm.tile([64, 4], F32, tag="sm")
        nc.tensor.transpose(ut_ps[:], u_sb[:], ident[0:4, 0:4])
        sm_t = ew.tile([64, 4], F32, tag="smt")
        softmax4(ut_ps[:], 64, sm_t[:])
        tb_ps = ps_sm.tile([4, 64], F32, tag="sm")
        nc.tensor.transpose(tb_ps[:], sm_t[:], ident[0:64, 0:64])
        nc.scalar.copy(uct[:, (m - 1) * 64:m * 64], tb_ps[:])

    # ---- G: G[r, h, w] = U3T[r,h]*U2T[r,w] (DVE, first half gates pt0) ----
    Gv = G[:].rearrange("p (h w) -> p h w", h=64)
    nc.vector.tensor_tensor(
        Gv[:, 0:32, :],
        uct[0:4, 64:96][:, :, None].broadcast_to([4, 32, 64]),
        uct[0:4, 0:64][:, None, :].broadcast_to([4, 32, 64]), op=ALU.mult)
    nc.vector.tensor_tensor(
        Gv[:, 32:64, :],
        uct[0:4, 96:128][:, :, None].broadcast_to([4, 32, 64]),
        uct[0:4, 0:64][:, None, :].broadcast_to([4, 32, 64]), op=ALU.mult)

    # ---- MT rows 0-3 = (Wr @ U1 diag(lam))^T ----
    for mm in range(2):
        m_ps = ps_sm.tile([128, 4], F32, tag="sm")
        for kk in range(2):
            nc.tensor.matmul(m_ps[:], wrt_sb[:, kk, mm, :], U1n[:, kk, :],
                             start=(kk == 0), stop=(kk == 1))
        m_sb = ew.tile([128, 4], F32, tag="msb")
        nc.scalar.copy(m_sb[:], m_ps[:])
        mt_ps = ps_sm.tile([4, 128], F32, tag="sm")
        nc.tensor.transpose(mt_ps[:], m_sb[:], ident[:])
        nc.vector.tensor_scalar(MT[:, mm * 128:(mm + 1) * 128], mt_ps[:],
                                lam_sb[:], None, op0=ALU.mult)

    # ---- spectral attention ----
    gag = consts.tile([128, 4], F32)  # [ga_ct0, ga_ct1, gm_ct0, gm_ct1]
    for ct in range(2):
        f_ps = ps_sm.tile([128, 128], F32, tag="sm")
        nc.tensor.matmul(f_ps[:], U1T[:, ct * 128:(ct + 1) * 128], uct[:],
                         start=True, stop=True)
        nc.vector.tensor_reduce(gag[:, ct:ct + 1], f_ps[:], axis=AX.X,
                                op=ALU.add)
        nc.vector.tensor_reduce(gag[:, 2 + ct:3 + ct], f_ps[:], axis=AX.X,
                                op=ALU.max)
    spectral = consts.tile([128, 2], F32)
    for mm in range(2):
        sp_ps = ps_sm.tile([128, 1], F32, tag="sm")
        for kk in range(4):
            nc.tensor.matmul(sp_ps[:], wsc_sb[:, kk, mm, :],
                             gag[:, kk:kk + 1], start=(kk == 0),
                             stop=(kk == 3))
        stmp = ew.tile([128, 1], F32, tag="stmp")
        nc.scalar.activation(stmp[:], sp_ps[:], AF.Sigmoid,
                             bias=bsc_sb[:, mm:mm + 1])
        nc.scalar.activation(spectral[:, mm:mm + 1], stmp[:], AF.Sigmoid)

    # ---- final elementwise stage ----
    # fused = D*sigc + Q;  cp_recon = (rc + br)*sigc + Fm; sigc = spc*sig
    fo_v = fused_o.rearrange("(c p) h w -> p c h w", p=128)
    co_v = cpr_o.rearrange("(c p) h w -> p c h w", p=128)
    for pt in range(8):
        sp_ps = ps_spat.tile([128, 512], F32, tag="spat")
        nc.tensor.matmul(sp_ps[:], ones4[:], G[:, pt * 512:(pt + 1) * 512],
                         start=True, stop=True)
        sig = ew.tile([128, 512], BF16, tag="sig")
        nc.scalar.activation(sig[:], sp_ps[:], AF.Sigmoid,
                             scale=float(ws), bias=float(bs))
        rcp = ps_rec.tile([128, 2, 512], F32, tag="rc")
        for ct in range(2):
            nc.tensor.matmul(rcp[:, ct], MT[:, ct * 128:(ct + 1) * 128],
                             G[:, pt * 512:(pt + 1) * 512], start=True,
                             stop=False)
            nc.tensor.matmul(rcp[:, ct],
                             brrow_sb[0:1, ct * 128:(ct + 1) * 128],
                             ones512b[0:1, :], start=False, stop=True)
        R = ew.tile([128, 2, 512], BF16, tag="R")
        E = ew.tile([128, 2, 512], BF16, tag="E")
        E2 = ew.tile([128, 2, 512], BF16, tag="E2")
        for ct in range(2):
            nc.scalar.copy(R[:, ct], rcp[:, ct])
            nc.vector.scalar_tensor_tensor(
                E[:, ct], Dall[:, pt, ct], spectral[:, ct:ct + 1], sig[:],
                op0=ALU.mult, op1=ALU.mult)
            nc.vector.scalar_tensor_tensor(
                E2[:, ct], R[:, ct], spectral[:, ct:ct + 1], sig[:],
                op0=ALU.mult, op1=ALU.mult)
        fu = outr.tile([128, 2, 512], BF16, tag="fu")
        nc.vector.tensor_tensor(fu[:], E[:], Qall[:, pt], op=ALU.add)
        cp = outr.tile([128, 2, 512], BF16, tag="cp")
        nc.gpsimd.tensor_tensor(cp[:], E2[:], fm_sb[:, pt], op=ALU.add)
        for ct in range(2):
            nc.sync.dma_start(
                fo_v[:, ct, pt * 8:(pt + 1) * 8, :],
                fu[:, ct].rearrange("p (h w) -> p h w", h=8))
            nc.sync.dma_start(
                co_v[:, ct, pt * 8:(pt + 1) * 8, :],
                cp[:, ct].rearrange("p (h w) -> p h w", h=8))
    ctx.close()


def _prep_weights(W3, b3, Wa1, ba1, Wa2, ba2, Wa3, ba3, Wu, bu, Wr, br,
                  Wsa, bsa, Wsm, bsm, lam):
    f = np.float32
    bf = ml_dtypes.bfloat16
    # w3t[p, ch, kt, t, co'] = W3[ch*128+co', kt*128+p, dy, dx]
    w3t = np.ascontiguousarray(
        W3.reshape(2, 128, 4, 128, 9).transpose(3, 0, 2, 4, 1)).astype(bf)
    b3h = b3.reshape(2, 128).T
    # wrt[p, kk, mm, m] = Wr[mm*128+m, kk*128+p]
    wrt = Wr.reshape(2, 128, 2, 128).transpose(3, 2, 0, 1).reshape(128, 512)
    brn = br.reshape(2, 128).T
    # wsc[p, kk, mm, m]: kk<2 -> Wsa/128 (mean folded), kk>=2 -> Wsm
    wsa_r = (Wsa / 128.0).reshape(2, 128, 2, 128).transpose(3, 2, 0, 1)
    wsm_r = Wsm.reshape(2, 128, 2, 128).transpose(3, 2, 0, 1)
    wsc = np.concatenate([wsa_r, wsm_r], axis=1).reshape(128, 1024)
    bsc = (bsa + bsm).reshape(2, 128).T
    smallw = np.ascontiguousarray(np.concatenate(
        [wrt, wsc, b3h, brn, bsc], axis=1)).astype(f)
    # tiny[r, :] = [bum(3) | lam(1) | wum(24, row0 only) | wub1(12, row0)]
    bum = np.stack([Wu @ ba1 + bu, Wu @ ba2 + bu, Wu @ ba3 + bu], axis=1)
    wum = np.stack([(Wu @ Wa1).T, (Wu @ Wa2).T, (Wu @ Wa3).T], axis=0)
    wuwa1 = Wu @ Wa1
    wub1 = np.concatenate([wuwa1[:, 0], wuwa1[:, 1], Wu @ ba1 + bu])
    tiny = np.zeros((4, 52), f)
    tiny[:, 0:3] = bum
    tiny[:, 3] = np.asarray(lam).reshape(4)
    tiny[0, 4:28] = wum.reshape(24)
    tiny[0, 28:40] = wub1
    tiny[0, 40:52] = bum.T.reshape(12)   # bumrow: [m, r] flattened
    brrow = np.ascontiguousarray(br.reshape(1, 256)).astype(bf)
    return dict(w3t=w3t, smallw=smallw, tiny=tiny, brrow=brrow)


_CACHE = {}


def kernel(frm_feat, other_feat, W3, b3, Wa1, ba1, Wa2, ba2, Wa3, ba3,
           Wu, bu, Wr, br, ws, bs, Wsa, bsa, Wsm, bsm, alpha, lam,
           _trace=False, _tmpdir=None):
    frm_feat = np.asarray(frm_feat, np.float32)
    other_feat = np.asarray(other_feat, np.float32)
    key = (float(alpha), float(ws), float(bs))
    if key not in _CACHE:
        _CACHE[key] = build_program(float(alpha), float(ws), float(bs))
    nc = _CACHE[key]

    wd = _prep_weights(np.asarray(W3), np.asarray(b3), np.asarray(Wa1),
                       np.asarray(ba1), np.asarray(Wa2), np.asarray(ba2),
                       np.asarray(Wa3), np.asarray(ba3), np.asarray(Wu),
                       np.asarray(bu), np.asarray(Wr), np.asarray(br),
                       np.asarray(Wsa), np.asarray(bsa), np.asarray(Wsm),
                       np.asarray(bsm), np.asarray(lam))

    in_maps = []
    for b_i in range(NCORES):
        m = dict(wd)
        m["frm"] = np.ascontiguousarray(frm_feat[b_i])
        m["oth"] = np.ascontiguousarray(other_feat[b_i])
        in_maps.append(m)

    res = bass_utils.run_bass_kernel_spmd(
        nc, in_maps, core_ids=list(range(NCORES)), trace=_trace,
        tmpdir=_tmpdir)
    fused = np.stack([np.asarray(res.results[i]["fused"])
                      for i in range(NCORES)]).astype(np.float32)
    cpr = np.stack([np.asarray(res.results[i]["cpr"])
                    for i in range(NCORES)]).astype(np.float32)
    kernel._last_exec_time_ns = res.exec_time_ns
    kernel._last_results = res
    return fused, cpr
